# revision 2
# baseline (speedup 1.0000x reference)
"""Trainium2 Bass kernel for nn_AttentionEnhancedBiLSTM (8 NeuronCores, SPMD).

Math (from the reference):
    x  = inputs[:, -1, :]                        # [B=1024, E=1024]
    af = softmax((x Wq^T)(x Wk^T)^T / 32) (x Wv^T) Wo^T + bo     (fwd dir)
    h_f = sigmoid(o) * tanh(sigmoid(i) * tanh(g)),  gates = (af+x) W_ih^T + b
    backward: same with xr = x[:, ::-1] and its own weights; output keeps the
    CELL state c_b = sigmoid(i)*tanh(g).
    out = concat([h_f, c_b], -1)                 # [1024, 1024]

Sharding: batch-sharded 8 ways (128 rows/core). Attention mixes the batch, so
each core computes k^T and v for its own rows and the full k^T/v are formed
with one AllGather per direction; everything else is local. Weights are
replicated (cheaper than TP: activation exchanges through DRAM cost more HBM
than the weight replication saves).

Schedule: kv_f -> AG_f -> kv_b -> AG_b -> q_f -> attn_f -> q_b -> attn_b, so
both collectives fly under local compute. Matmul operands are float32r
(single-pass fp32, full PE rate at moving dim >= 256); activations feeding a
matmul as the stationary operand are transposed on the PE.
"""

import numpy as np

import concourse.bass as bass
import concourse.mybir as mybir
import concourse.tile as tile
from concourse import bacc
from concourse.bass_utils import run_bass_kernel_spmd
from concourse.masks import make_identity

N_CORES = 8
B, T, E, H = 1024, 128, 1024, 512
BS = B // N_CORES          # 128 batch rows per core
NE = E // 128              # 8 e-chunks
F32 = mybir.dt.float32
FMM = mybir.dt.float32r


class _Dir:
    """Per-direction build state."""

    def __init__(self, d, ext, compute_h):
        self.d = d
        self.ext = ext
        self.compute_h = compute_h
        self.G = 3 * H if compute_h else 2 * H


def _emit(tc, nc, sb, ps, dram, ident, ones, dirs, out_sb, with_attn_bias):

    def mm_full(st, w_ext, b_ext, name, dma_eng):
        """psum[128, E] = x_shard @ W^T (+ b)  (lhsT = xT chunks, rhs = w)."""
        acc = ps.tile([128, E], F32, name=f"ps_{name}", tag="mm")
        for ec in range(NE):
            wt = sb.tile([128, E], FMM, name=f"w_{name}_{ec}", tag="w")
            dma_eng.dma_start(wt[:], w_ext[ec * 128:(ec + 1) * 128, :])
            for n in range(E // 512):
                nc.tensor.matmul(
                    acc[:, n * 512:(n + 1) * 512],
                    st.xT[:, ec * 128:(ec + 1) * 128],
                    wt[:, n * 512:(n + 1) * 512],
                    start=(ec == 0), stop=(ec == NE - 1 and not with_attn_bias),
                )
        if with_attn_bias:
            bt = sb.tile([1, E], FMM, name=f"b_{name}", tag="bias")
            nc.sync.dma_start(bt[:], b_ext[:])
            for n in range(E // 512):
                nc.tensor.matmul(
                    acc[:, n * 512:(n + 1) * 512],
                    ones[0:1, :],
                    bt[0:1, n * 512:(n + 1) * 512],
                    start=False, stop=True,
                )
        return acc

    def transpose_1024(src_sb, dst_name, dst_tag):
        """[128, 1024] natural -> [128, 1024] transposed-chunks via PE."""
        out = sb.tile([128, E], FMM, name=dst_name, tag=dst_tag)
        for half in range(2):
            tp = ps.tile([128, 512], FMM, name=f"tp_{dst_name}_{half}", tag="tp")
            for i in range(4):
                j = half * 4 + i
                nc.tensor.transpose(
                    tp[:, i * 128:(i + 1) * 128],
                    src_sb[:, j * 128:(j + 1) * 128],
                    ident[:],
                )
            nc.vector.tensor_copy(out[:, half * 512:(half + 1) * 512], tp[:])
        return out

    # ---- phase A (both dirs): local k^T and v shard + AllGather ----------
    for st in dirs:
        d, ext = st.d, st.ext
        st.xT = sb.tile([128, E], FMM, name=f"xT_{d}", tag=f"xT_{d}")
        nc.sync.dma_start(st.xT[:], ext["xT"].rearrange("(n p) m -> p n m", p=128))

        # bounce layout: rows [0:128) = k^T chunks as [p, jc*128+b];
        #                rows [128:256) = v natural [b, j]
        st.bounce_in = dram.tile([2 * BS, E], FMM, name=f"bin_{d}")
        st.bounce_out = dram.tile([N_CORES, 2 * BS, E], FMM, name=f"bout_{d}",
                                  addr_space="Shared")

        k_ps = mm_full(st, ext["wk"], ext["bk"], f"k{d}", nc.sync)
        k_sb = sb.tile([128, E], FMM, name=f"k_{d}", tag="act")
        for n in range(2):
            nc.vector.tensor_copy(k_sb[:, n * 512:(n + 1) * 512],
                                  k_ps[:, n * 512:(n + 1) * 512])
        kT = transpose_1024(k_sb, f"kT_{d}", "act2")
        nc.scalar.dma_start(st.bounce_in[0:BS, :], kT[:])

        v_ps = mm_full(st, ext["wv"], ext["bv"], f"v{d}", nc.scalar)
        v_sb = sb.tile([128, E], FMM, name=f"v_{d}", tag="act")
        for n in range(2):
            nc.vector.tensor_copy(v_sb[:, n * 512:(n + 1) * 512],
                                  v_ps[:, n * 512:(n + 1) * 512])
        nc.scalar.dma_start(st.bounce_in[BS:2 * BS, :], v_sb[:])

        nc.gpsimd.collective_compute(
            "AllGather",
            mybir.AluOpType.bypass,
            replica_groups=[list(range(N_CORES))],
            ins=[st.bounce_in.opt()],
            outs=[st.bounce_out.opt()],
        )

    # ---- phases B + C per direction --------------------------------------
    for st in dirs:
        d, ext, G = st.d, st.ext, st.G

        # B: q^T (overlaps the collectives)
        q_ps = mm_full(st, ext["wq"], ext["bq"], f"q{d}", nc.sync)
        q_sb = sb.tile([128, E], FMM, name=f"q_{d}", tag="act")
        for n in range(2):
            nc.vector.tensor_copy(q_sb[:, n * 512:(n + 1) * 512],
                                  q_ps[:, n * 512:(n + 1) * 512])
        qT = transpose_1024(q_sb, f"qT_{d}", f"qT_{d}")

        # C: attention + LSTM cell
        # k^T full / v full, g-major free layout: [128, g*1024 + (jc*128+b | j)]
        kT_full = sb.tile([128, NE * E], FMM, name=f"kTf_{d}", tag="kT_full")
        v_full = sb.tile([128, NE * E], FMM, name=f"vf_{d}", tag="v_full")
        for g in range(N_CORES):
            nc.scalar.dma_start(kT_full[:, g * E:(g + 1) * E],
                                st.bounce_out[g, 0:BS, :])
            nc.sync.dma_start(v_full[:, g * E:(g + 1) * E],
                                st.bounce_out[g, BS:2 * BS, :])

        # scores[b, b'] += qT[jc]^T @ kT[jc, b']; b' = g*128 + b_local
        kT_g = kT_full.rearrange("p (g x) -> p g x", g=N_CORES)
        scores = ps.tile([128, B], F32, name=f"scores_{d}", tag="mm")
        for jc in range(NE):
            for n in range(B // 512):
                nc.tensor.matmul(
                    scores[:, n * 512:(n + 1) * 512],
                    qT[:, jc * 128:(jc + 1) * 128],
                    kT_g[:, 4 * n:4 * (n + 1), jc * 128:(jc + 1) * 128],
                    start=(jc == 0), stop=(jc == NE - 1),
                )

        # softmax along free axis (scores pre-scaled by 1/32 via wq)
        negmax = sb.tile([128, 1], F32, name=f"negmax_{d}", tag="stat")
        nc.vector.reduce_max(out=negmax[:], in_=scores[:],
                             axis=mybir.AxisListType.X, negate=True)
        p_sb = sb.tile([128, B], FMM, name=f"p_{d}", tag="act")
        rowsum = sb.tile([128, 1], F32, name=f"rowsum_{d}", tag="stat")
        nc.scalar.activation(p_sb[:], scores[:], mybir.ActivationFunctionType.Exp,
                             bias=negmax[:], scale=1.0, accum_out=rowsum[:])
        rinv = sb.tile([128, 1], F32, name=f"rinv_{d}", tag="stat")
        nc.vector.reciprocal(rinv[:], rowsum[:])

        pT = transpose_1024(p_sb, f"pT_{d}", "act2")
        av_ps = ps.tile([128, E], F32, name=f"av_{d}", tag="mm")
        for bc in range(NE):
            for n in range(E // 512):
                nc.tensor.matmul(
                    av_ps[:, n * 512:(n + 1) * 512],
                    pT[:, bc * 128:(bc + 1) * 128],
                    v_full[:, bc * E + n * 512: bc * E + (n + 1) * 512],
                    start=(bc == 0), stop=(bc == NE - 1),
                )
        av_sb = sb.tile([128, E], FMM, name=f"avn_{d}", tag="act")
        nc.vector.tensor_scalar_mul(av_sb[:], av_ps[:], rinv[:])

        avT = transpose_1024(av_sb, f"avT_{d}", "act2")
        af_ps = ps.tile([128, E], F32, name=f"af_{d}", tag="mm")
        for jc in range(NE):
            wot = sb.tile([128, E], FMM, name=f"wo_{d}_{jc}", tag="w")
            nc.sync.dma_start(wot[:], ext["wo"][jc * 128:(jc + 1) * 128, :])
            for n in range(E // 512):
                nc.tensor.matmul(
                    af_ps[:, n * 512:(n + 1) * 512],
                    avT[:, jc * 128:(jc + 1) * 128],
                    wot[:, n * 512:(n + 1) * 512],
                    start=(jc == 0), stop=(jc == NE - 1 and not with_attn_bias),
                )
        if with_attn_bias:
            bo_sb = sb.tile([1, E], FMM, name=f"bo_{d}", tag="bias")
            nc.sync.dma_start(bo_sb[:], ext["bo"][:])
            for n in range(E // 512):
                nc.tensor.matmul(
                    af_ps[:, n * 512:(n + 1) * 512],
                    ones[0:1, :],
                    bo_sb[0:1, n * 512:(n + 1) * 512],
                    start=False, stop=True,
                )

        # lstm_in = af + x (natural), then transpose for the gates matmul
        x_sb = sb.tile([128, E], F32, name=f"x_{d}", tag="act")
        nc.sync.dma_start(x_sb[:], ext["x"][:])
        lstm_sb = sb.tile([128, E], FMM, name=f"lstm_{d}", tag="act")
        nc.vector.tensor_add(lstm_sb[:], af_ps[:], x_sb[:])
        lstmT = transpose_1024(lstm_sb, f"lstmT_{d}", "act2")

        gates = ps.tile([128, G], F32, name=f"gates_{d}", tag="mm")
        for ec in range(NE):
            wih = sb.tile([128, G], FMM, name=f"wih_{d}_{ec}", tag="w")
            nc.scalar.dma_start(wih[:], ext["wih"][ec * 128:(ec + 1) * 128, :])
            for n in range(G // 512):
                nc.tensor.matmul(
                    gates[:, n * 512:(n + 1) * 512],
                    lstmT[:, ec * 128:(ec + 1) * 128],
                    wih[:, n * 512:(n + 1) * 512],
                    start=(ec == 0), stop=False,
                )
        bih = sb.tile([1, G], FMM, name=f"bih_{d}", tag="bias")
        nc.sync.dma_start(bih[:], ext["bih"][:])
        for n in range(G // 512):
            nc.tensor.matmul(
                gates[:, n * 512:(n + 1) * 512],
                ones[0:1, :],
                bih[0:1, n * 512:(n + 1) * 512],
                start=False, stop=True,
            )

        # gate nonlinearities; c = sig(i)*tanh(g); fwd also h = sig(o)*tanh(c)
        Sig = mybir.ActivationFunctionType.Sigmoid
        Tanh = mybir.ActivationFunctionType.Tanh
        si = sb.tile([128, H], F32, name=f"si_{d}", tag="gate")
        nc.scalar.activation(si[:], gates[:, 0:H], Sig)
        tg = sb.tile([128, H], F32, name=f"tg_{d}", tag="gate")
        nc.scalar.activation(tg[:], gates[:, H:2 * H], Tanh)
        if st.compute_h:
            cst = sb.tile([128, H], F32, name=f"c_{d}", tag="gate")
            nc.vector.tensor_mul(cst[:], si[:], tg[:])
            tc_ = sb.tile([128, H], F32, name=f"tc_{d}", tag="gate")
            nc.scalar.activation(tc_[:], cst[:], Tanh)
            so = sb.tile([128, H], F32, name=f"so_{d}", tag="gate")
            nc.scalar.activation(so[:], gates[:, 2 * H:3 * H], Sig)
            nc.vector.tensor_mul(out_sb[:, 0:H], so[:], tc_[:])
        else:
            nc.vector.tensor_mul(out_sb[:, H:2 * H], si[:], tg[:])


def build_nc(with_attn_bias=False):
    nc = bacc.Bacc("TRN2", target_bir_lowering=False, debug=False,
                   num_devices=N_CORES)

    def din(name, shape, dt=FMM):
        return nc.dram_tensor(name, shape, dt, kind="ExternalInput").ap()

    ext = {}
    for d in ("f", "b"):
        ext[d] = {
            "xT": din(f"xT_{d}", [E, BS]),
            "x": din(f"x_{d}", [BS, E], F32),
            "wq": din(f"wq_{d}", [E, E]),
            "wk": din(f"wk_{d}", [E, E]),
            "wv": din(f"wv_{d}", [E, E]),
            "wo": din(f"wo_{d}", [E, E]),
            "bq": din(f"bq_{d}", [1, E]),
            "bk": din(f"bk_{d}", [1, E]),
            "bv": din(f"bv_{d}", [1, E]),
            "bo": din(f"bo_{d}", [1, E]),
        }
    ext["f"]["wih"] = din("wih_f", [E, 3 * H])
    ext["f"]["bih"] = din("bih_f", [1, 3 * H])
    ext["b"]["wih"] = din("wih_b", [E, 2 * H])
    ext["b"]["bih"] = din("bih_b", [1, 2 * H])
    out_ext = nc.dram_tensor("out", [BS, 2 * H], F32, kind="ExternalOutput").ap()

    with tile.TileContext(nc) as tc:
        with (
            tc.tile_pool(name="sb", bufs=1) as sb_pool,
            tc.tile_pool(name="ps", bufs=1, space="PSUM") as ps_pool,
            tc.tile_pool(name="dram", bufs=1, space="DRAM") as dram_pool,
        ):
            class P:
                def __init__(self, pool, defaults):
                    self.pool, self.defaults = pool, defaults

                def tile(self, shape, dtype, name=None, tag=""):
                    bufs = self.defaults.get(tag, 1)
                    return self.pool.tile(shape, dtype, name=name, tag=tag,
                                          bufs=bufs)

            sb = P(sb_pool, {"w": 5, "act": 4, "act2": 3, "bias": 2,
                             "gate": 6, "stat": 4})
            ps = P(ps_pool, {"mm": 2, "tp": 2})

            class D:
                def tile(self, shape, dtype, name=None, addr_space="Local"):
                    return dram_pool.tile(shape, dtype, name=name,
                                          addr_space=addr_space)

            dram = D()

            ident_f = sb_pool.tile([128, 128], F32, name="ident_f", tag="ident_f")
            make_identity(nc, ident_f)
            ones_f = sb_pool.tile([1, 128], F32, name="ones_f", tag="ones_f")
            nc.gpsimd.memset(ones_f[:], 1.0)
            ident = sb_pool.tile([128, 128], FMM, name="ident", tag="ident")
            nc.vector.tensor_copy(ident[:], ident_f[:])
            ones = sb_pool.tile([1, 128], FMM, name="ones", tag="ones")
            nc.vector.tensor_copy(ones[:], ones_f[:])

            out_sb = sb_pool.tile([BS, 2 * H], F32, name="out_sb", tag="out")

            dirs = [_Dir("f", ext["f"], True), _Dir("b", ext["b"], False)]
            _emit(tc, nc, sb, ps, dram, ident, ones, dirs, out_sb,
                  with_attn_bias)

            nc.sync.dma_start(out_ext[:], out_sb[:])

    nc.compile()
    return nc


_NC_CACHE = {}


def _get_nc(with_attn_bias=False):
    if with_attn_bias not in _NC_CACHE:
        _NC_CACHE[with_attn_bias] = build_nc(with_attn_bias)
    return _NC_CACHE[with_attn_bias]


def _prep_host(inputs, Wqkv, bqkv, Wo, bo, W_ih, b_ih, b_hh, flip):
    """Per-direction host-side tensors (shared across cores except x shards)."""
    c = np.ascontiguousarray
    x = inputs
    if flip:
        x = x[:, ::-1]
    wq = c(Wqkv[0:E].T.astype(np.float32) / 32.0)   # fold 1/sqrt(E)
    wk = c(Wqkv[E:2 * E].T.astype(np.float32))
    wv = c(Wqkv[2 * E:3 * E].T.astype(np.float32))
    bq = c(bqkv[0:E].reshape(1, E) / 32.0)
    bk = c(bqkv[E:2 * E].reshape(1, E))
    bv = c(bqkv[2 * E:3 * E].reshape(1, E))
    wo_t = c(Wo.T)
    bo_r = c(bo.reshape(1, E))
    blstm = b_ih + b_hh
    if flip:    # backward: only i and g gates are used
        wih = c(np.concatenate([W_ih[0:H], W_ih[2 * H:3 * H]], axis=0).T)
        bih = c(np.concatenate([blstm[0:H], blstm[2 * H:3 * H]]).reshape(1, -1))
    else:       # forward: i, g, o
        wih = c(np.concatenate([W_ih[0:H], W_ih[2 * H:3 * H],
                                W_ih[3 * H:4 * H]], axis=0).T)
        bih = c(np.concatenate([blstm[0:H], blstm[2 * H:3 * H],
                                blstm[3 * H:4 * H]]).reshape(1, -1))
    return x, dict(wq=wq, wk=wk, wv=wv, wo=wo_t, bq=bq, bk=bk, bv=bv,
                   bo=bo_r, wih=wih, bih=bih)


def kernel(inputs, Wqkv_f, bqkv_f, Wo_f, bo_f, W_ih_f, b_ih_f, b_hh_f,
           Wqkv_b, bqkv_b, Wo_b, bo_b, W_ih_b, b_ih_b, b_hh_b):
    inputs = np.asarray(inputs, dtype=np.float32)
    x_last = np.ascontiguousarray(inputs[:, -1, :])          # [B, E]

    xf, shared_f = _prep_host(x_last, np.asarray(Wqkv_f), np.asarray(bqkv_f),
                              np.asarray(Wo_f), np.asarray(bo_f),
                              np.asarray(W_ih_f), np.asarray(b_ih_f),
                              np.asarray(b_hh_f), flip=False)
    xb, shared_b = _prep_host(x_last, np.asarray(Wqkv_b), np.asarray(bqkv_b),
                              np.asarray(Wo_b), np.asarray(bo_b),
                              np.asarray(W_ih_b), np.asarray(b_ih_b),
                              np.asarray(b_hh_b), flip=True)

    with_attn_bias = bool(
        np.any(np.asarray(bqkv_f)) or np.any(np.asarray(bo_f))
        or np.any(np.asarray(bqkv_b)) or np.any(np.asarray(bo_b)))

    c = np.ascontiguousarray
    in_maps = []
    for ci in range(N_CORES):
        rows = slice(ci * BS, (ci + 1) * BS)
        m = {"xT_f": c(xf[rows].T), "x_f": c(xf[rows]),
             "xT_b": c(xb[rows].T), "x_b": c(xb[rows])}
        for d, shared in (("f", shared_f), ("b", shared_b)):
            for k, v in shared.items():
                m[f"{k}_{d}"] = v
        in_maps.append(m)

    nc = _get_nc(with_attn_bias)
    res = run_bass_kernel_spmd(nc, in_maps, core_ids=list(range(N_CORES)))
    out = np.concatenate([res.results[ci]["out"] for ci in range(N_CORES)],
                         axis=0)
    return out.astype(np.float32)



# revision 10
# speedup vs baseline: 1.9996x; 1.9996x over previous
"""Trainium2 Bass kernel for nn_AttentionEnhancedBiLSTM (8 NeuronCores, SPMD).

Math (from the reference), per direction:
    x  = inputs[:, -1, :]                       # [B=1024, E=1024]
    scores = (x Wq^T)(x Wk^T)^T / 32
    af = softmax(scores) (x Wv^T) Wo^T + bo
    gates = (af + x) W_ih^T + b;  c = sig(i)*tanh(g);  h = sig(o)*tanh(c)
    out = concat([h_f, c_b], -1)   (backward direction uses xr = x[:, ::-1])

Factorization used here (exact in real arithmetic):
    scores = x Mq x^T,            Mq  = Wq^T Wk / 32
    gates  = rinv . (p @ x_all) @ W3 + x @ wih + bih_eff
        W3  = Wv^T Wo^T W_ih'^T   (W_ih' = used gate rows: fwd i,g,o; bwd i,g)
        wih = W_ih'^T
        bih_eff = (b_ih + b_hh)' + (bv Wo^T + bo) W_ih'^T   (softmax rows sum
        to 1, so bv/bo fold exactly; bk shifts scores per-row -> softmax
        invariant -> dropped; bq shifts per-column -> host vector vecb)
    The backward x-flip folds into the weights: Mq_b[::-1,::-1],
    W3_b[::-1,:], wih_b[::-1,:].  The device therefore needs only ONE
    AllGather of the raw x shard (transposed-pack + natural, bf16), shared by
    both directions, triggered at t=0 directly from the input DRAM tensor.

Precision: all attention matmuls bf16 (tested 1.5e-3 end-to-end rel err vs
2e-2 budget); the x @ wih gates branch stays float32r (bf16 there degrades to
1.2e-2).  Softmax runs without max-subtraction: |scores| <= ~6 so exp stays
comfortably inside f32/bf16 range.

Sharding: batch-sharded 8 ways (128 rows/core); weights replicated.
"""

import numpy as np
import ml_dtypes

import concourse.bass as bass
import concourse.mybir as mybir
import concourse.tile as tile
from concourse import bacc
from concourse.bass_utils import run_bass_kernel_spmd
from concourse.masks import make_identity

N_CORES = 8
B, T, E, H = 1024, 128, 1024, 512
BS = B // N_CORES          # 128 batch rows per core
NE = E // 128              # 8 e-chunks
GF = 3 * H                 # fwd gates i,g,o
GB = 2 * H                 # bwd gates i,g
F32 = mybir.dt.float32
FR = mybir.dt.float32r
BF = mybir.dt.bfloat16
BF_NP = ml_dtypes.bfloat16


def _emit(tc, nc, sb, ps, ident, ones32, onesb, ext, bounce_in, bounce_out,
          out_sb, with_attn_bias):
    Exp = mybir.ActivationFunctionType.Exp
    Sig = mybir.ActivationFunctionType.Sigmoid
    Tanh = mybir.ActivationFunctionType.Tanh
    Copy = mybir.ActivationFunctionType.Copy
    dirs = ("f", "b")
    G = {"f": GF, "b": GB}

    def transpose_1024(src, name, copy_on_scalar):
        """[128, 1024] bf16 natural -> chunk-transposed [128, 1024] bf16."""
        out = sb.tile([128, E], BF, name=name, tag=name)
        for half in range(2):
            tp = ps.tile([128, 512], BF, name=f"tp_{name}_{half}", tag="tp")
            for i in range(4):
                j = half * 4 + i
                nc.tensor.transpose(tp[:, i * 128:(i + 1) * 128],
                                    src[:, j * 128:(j + 1) * 128], ident[:])
            dst = out[:, half * 512:(half + 1) * 512]
            if copy_on_scalar:
                nc.scalar.activation(dst, tp[:], Copy)
            else:
                nc.vector.tensor_copy(dst, tp[:])
        return out

    # ---- t=0: single AllGather of the raw x shard (both layouts, bf16) ----
    # (collectives can't read IO tensors directly -> bounce through local DRAM)
    nc.gpsimd.dma_start(bounce_in.opt(), ext["xtp"])
    nc.gpsimd.collective_compute(
        "AllGather",
        mybir.AluOpType.bypass,
        replica_groups=[list(range(N_CORES))],
        ins=[bounce_in.opt()],
        outs=[bounce_out.opt()],
    )

    # ---- local loads (vector DMA queue) ----
    xt = sb.tile([128, E], BF, name="xt", tag="xt")
    nc.sync.dma_start(xt[:], ext["xtp"][0:BS, :])
    xT32 = sb.tile([128, E], FR, name="xT32", tag="xT32")
    nc.sync.dma_start(xT32[:], ext["xT32"].rearrange("(n p) m -> p n m",
                                                     p=128))

    # ---- pre-AG: xM = x @ Mq, then transpose (both dirs) ------------------
    xmT = {}
    for d in dirs:
        xm_ps = ps.tile([128, E], F32, name=f"xm_{d}", tag="mm")
        for ec in range(NE):
            mqt = sb.tile([128, E], BF, name=f"mq_{d}_{ec}", tag="w")
            nc.scalar.dma_start(mqt[:], ext[f"mq_{d}"][ec * 128:(ec + 1) * 128, :])
            for n in range(2):
                nc.tensor.matmul(
                    xm_ps[:, n * 512:(n + 1) * 512],
                    xt[:, ec * 128:(ec + 1) * 128],
                    mqt[:, n * 512:(n + 1) * 512],
                    start=(ec == 0), stop=(ec == NE - 1),
                )
        xm_sb = sb.tile([128, E], BF, name=f"xmsb_{d}", tag=f"xmsb_{d}")
        for n in range(2):
            nc.vector.tensor_copy(xm_sb[:, n * 512:(n + 1) * 512],
                                  xm_ps[:, n * 512:(n + 1) * 512])
        xmT[d] = transpose_1024(xm_sb, f"xmT_{d}", copy_on_scalar=False)

    # ---- pre-AG: gx = x @ wih + bih  (f32r branch, both dirs) -------------
    gx = {}
    for d in dirs:
        g = G[d]
        gx_ps = ps.tile([128, g], F32, name=f"gxps_{d}", tag="mm")
        for ec in range(NE):
            wt = sb.tile([128, g], FR, name=f"wih_{d}_{ec}", tag="w32")
            nc.sync.dma_start(wt[:], ext[f"wih_{d}"][ec * 128:(ec + 1) * 128, :])
            for n in range(g // 512):
                nc.tensor.matmul(
                    gx_ps[:, n * 512:(n + 1) * 512],
                    xT32[:, ec * 128:(ec + 1) * 128],
                    wt[:, n * 512:(n + 1) * 512],
                    start=(ec == 0), stop=False,
                )
        bih = sb.tile([1, g], FR, name=f"bih_{d}", tag="bias")
        nc.sync.dma_start(bih[:], ext[f"bih_{d}"][:])
        for n in range(g // 512):
            nc.tensor.matmul(
                gx_ps[:, n * 512:(n + 1) * 512],
                ones32[0:1, :],
                bih[0:1, n * 512:(n + 1) * 512],
                start=False, stop=(n == g // 512 - 1),
            )
        gx_sb = sb.tile([128, g], F32, name=f"gx_{d}", tag=f"gx_{d}")
        for n in range(g // 512):
            nc.vector.tensor_copy(gx_sb[:, n * 512:(n + 1) * 512],
                                  gx_ps[:, n * 512:(n + 1) * 512])
        gx[d] = gx_sb

    # ---- w3 (whole matrix, lands during the AG) ---------------------------
    w3 = {}
    for d in dirs:
        g = G[d]
        w3_sb = sb.tile([128, NE * g], BF, name=f"w3_{d}", tag=f"w3_{d}")
        nc.scalar.dma_start(w3_sb[:], ext[f"w3_{d}"].rearrange(
            "(n p) m -> p n m", p=128))
        w3[d] = w3_sb.rearrange("p (n m) -> p n m", n=NE)

    # ---- post-AG: gathered x in both layouts ------------------------------
    xTf = sb.tile([128, N_CORES * E], BF, name="xTf", tag="xTf")
    for g_ in range(N_CORES):
        nc.gpsimd.dma_start(xTf[:, g_ * E:(g_ + 1) * E],
                            bounce_out[g_, 0:BS, :])
    xnat = sb.tile([128, N_CORES * E], BF, name="xnat", tag="xnat")
    for g_ in range(N_CORES):
        nc.sync.dma_start(xnat[:, g_ * E:(g_ + 1) * E],
                          bounce_out[g_, BS:2 * BS, :])
    xTf4 = xTf.rearrange("p (g x) -> p g x", g=N_CORES)

    # ---- scores for both dirs (fills the softmax_f PE bubble) -------------
    sc_ps = {}
    for d in dirs:
        acc = ps.tile([128, B], F32, name=f"sc_{d}", tag="mm")
        for jc in range(NE):
            for n in range(B // 512):
                nc.tensor.matmul(
                    acc[:, n * 512:(n + 1) * 512],
                    xmT[d][:, jc * 128:(jc + 1) * 128],
                    xTf4[:, 4 * n:4 * (n + 1), jc * 128:(jc + 1) * 128],
                    start=(jc == 0),
                    stop=(jc == NE - 1 and not with_attn_bias),
                )
        if with_attn_bias:
            vb = sb.tile([1, B], BF, name=f"vecb_{d}", tag="bias")
            nc.sync.dma_start(vb[:], ext[f"vecb_{d}"][:])
            for n in range(B // 512):
                nc.tensor.matmul(
                    acc[:, n * 512:(n + 1) * 512],
                    onesb[0:1, :],
                    vb[0:1, n * 512:(n + 1) * 512],
                    start=False, stop=(n == B // 512 - 1),
                )
        sc_ps[d] = acc

    # ---- softmax (no max-subtraction; |scores| <= ~6) ---------------------
    pn = {}
    for d in dirs:
        rowsum = sb.tile([128, 1], F32, name=f"rowsum_{d}", tag="stat")
        p_sb = sb.tile([128, B], BF, name=f"p_{d}", tag=f"p_{d}")
        nc.scalar.activation(p_sb[:], sc_ps[d][:], Exp, accum_out=rowsum[:])
        rinv = sb.tile([128, 1], F32, name=f"rinv_{d}", tag="stat")
        nc.vector.reciprocal(rinv[:], rowsum[:])
        pn_sb = sb.tile([128, B], BF, name=f"pn_{d}", tag=f"pn_{d}")
        for n in range(2):
            nc.vector.tensor_scalar_mul(pn_sb[:, n * 512:(n + 1) * 512],
                                        p_sb[:, n * 512:(n + 1) * 512],
                                        rinv[:])
        pn[d] = pn_sb

    # ---- per dir: ax = pn @ x_all; gates = axT @ W3 + gx; nonlinearities --
    for d in dirs:
        g = G[d]
        pT = transpose_1024(pn[d], f"pT_{d}", copy_on_scalar=True)
        ax_ps = ps.tile([128, E], F32, name=f"ax_{d}", tag="mm")
        for bc in range(NE):
            for n in range(2):
                nc.tensor.matmul(
                    ax_ps[:, n * 512:(n + 1) * 512],
                    pT[:, bc * 128:(bc + 1) * 128],
                    xnat[:, bc * E + n * 512: bc * E + (n + 1) * 512],
                    start=(bc == 0), stop=(bc == NE - 1),
                )
        ax_sb = sb.tile([128, E], BF, name=f"axsb_{d}", tag=f"axsb_{d}")
        for n in range(2):
            nc.vector.tensor_copy(ax_sb[:, n * 512:(n + 1) * 512],
                                  ax_ps[:, n * 512:(n + 1) * 512])
        axT = transpose_1024(ax_sb, f"axT_{d}", copy_on_scalar=True)

        gp_ps = ps.tile([128, g], F32, name=f"gp_{d}", tag="mm")
        for ec in range(NE):
            for n in range(g // 512):
                nc.tensor.matmul(
                    gp_ps[:, n * 512:(n + 1) * 512],
                    axT[:, ec * 128:(ec + 1) * 128],
                    w3[d][:, ec, n * 512:(n + 1) * 512],
                    start=(ec == 0), stop=(ec == NE - 1),
                )
        gt = sb.tile([128, g], F32, name=f"gt_{d}", tag=f"gt_{d}")
        for n in range(g // 512):
            nc.vector.tensor_add(gt[:, n * 512:(n + 1) * 512],
                                 gp_ps[:, n * 512:(n + 1) * 512],
                                 gx[d][:, n * 512:(n + 1) * 512])

        si = sb.tile([128, H], F32, name=f"si_{d}", tag="gate")
        nc.scalar.activation(si[:], gt[:, 0:H], Sig)
        tg = sb.tile([128, H], F32, name=f"tg_{d}", tag="gate")
        nc.scalar.activation(tg[:], gt[:, H:2 * H], Tanh)
        if d == "f":
            cst = sb.tile([128, H], F32, name="c_f", tag="gate")
            nc.vector.tensor_mul(cst[:], si[:], tg[:])
            tc_ = sb.tile([128, H], F32, name="tc_f", tag="gate")
            nc.scalar.activation(tc_[:], cst[:], Tanh)
            so = sb.tile([128, H], F32, name="so_f", tag="gate")
            nc.scalar.activation(so[:], gt[:, 2 * H:3 * H], Sig)
            nc.vector.tensor_mul(out_sb[:, 0:H], so[:], tc_[:])
        else:
            nc.vector.tensor_mul(out_sb[:, H:2 * H], si[:], tg[:])


def build_nc(with_attn_bias=False):
    nc = bacc.Bacc("TRN2", target_bir_lowering=False, debug=False,
                   num_devices=N_CORES)

    def din(name, shape, dt):
        return nc.dram_tensor(name, shape, dt, kind="ExternalInput").ap()

    ext = {
        "xtp": din("xtp", [2 * BS, E], BF),
        "xT32": din("xT32", [E, BS], FR),
    }
    for d, g in (("f", GF), ("b", GB)):
        ext[f"mq_{d}"] = din(f"mq_{d}", [E, E], BF)
        ext[f"w3_{d}"] = din(f"w3_{d}", [E, g], BF)
        ext[f"wih_{d}"] = din(f"wih_{d}", [E, g], FR)
        ext[f"bih_{d}"] = din(f"bih_{d}", [1, g], FR)
        if with_attn_bias:
            ext[f"vecb_{d}"] = din(f"vecb_{d}", [1, B], BF)
    out_ext = nc.dram_tensor("out", [BS, 2 * H], F32, kind="ExternalOutput").ap()

    with tile.TileContext(nc) as tc:
        with (
            tc.tile_pool(name="sb", bufs=1) as sb_pool,
            tc.tile_pool(name="ps", bufs=1, space="PSUM") as ps_pool,
            tc.tile_pool(name="dram", bufs=1, space="DRAM") as dram_pool,
        ):
            class P:
                def __init__(self, pool, defaults):
                    self.pool, self.defaults = pool, defaults

                def tile(self, shape, dtype, name=None, tag=""):
                    bufs = self.defaults.get(tag, 1)
                    return self.pool.tile(shape, dtype, name=name, tag=tag,
                                          bufs=bufs)

            sb = P(sb_pool, {"w": 4, "w32": 3, "bias": 2, "gate": 6,
                             "stat": 4})
            ps = P(ps_pool, {"mm": 2, "tp": 2})

            identf = sb_pool.tile([128, 128], F32, name="identf", tag="identf")
            make_identity(nc, identf)
            ident = sb_pool.tile([128, 128], BF, name="ident", tag="ident")
            nc.vector.tensor_copy(ident[:], identf[:])
            onesf = sb_pool.tile([1, 128], F32, name="onesf", tag="onesf")
            nc.gpsimd.memset(onesf[:], 1.0)
            ones32 = sb_pool.tile([1, 128], FR, name="ones32", tag="ones32")
            nc.vector.tensor_copy(ones32[:], onesf[:])
            onesb = sb_pool.tile([1, 128], BF, name="onesb", tag="onesb")
            nc.vector.tensor_copy(onesb[:], onesf[:])

            bounce_in = dram_pool.tile([2 * BS, E], BF, name="bounce_in")
            bounce_out = dram_pool.tile([N_CORES, 2 * BS, E], BF,
                                        name="bounce", addr_space="Shared")
            out_sb = sb_pool.tile([BS, 2 * H], F32, name="out_sb", tag="out")

            _emit(tc, nc, sb, ps, ident, ones32, onesb, ext, bounce_in,
                  bounce_out, out_sb, with_attn_bias)

            nc.gpsimd.dma_start(out_ext[:], out_sb[:])

    nc.compile()
    return nc


_NC_CACHE = {}


def _get_nc(with_attn_bias=False):
    if with_attn_bias not in _NC_CACHE:
        _NC_CACHE[with_attn_bias] = build_nc(with_attn_bias)
    return _NC_CACHE[with_attn_bias]


def _prep_dir(x, Wqkv, bqkv, Wo, bo, W_ih, b_ih, b_hh, flip, want_h):
    """Host-side weight folds for one direction (shared across cores)."""
    c = np.ascontiguousarray
    Wq, Wk, Wv = Wqkv[0:E], Wqkv[E:2 * E], Wqkv[2 * E:3 * E]
    idx = np.r_[0:H, 2 * H:3 * H, 3 * H:4 * H] if want_h else np.r_[0:H, 2 * H:3 * H]
    Wih = W_ih[idx]                                     # [G, E]
    Mq = (Wq.T @ Wk) / 32.0                             # [E, E]
    WoWih = Wo.T @ Wih.T                                # [E, G]
    W3 = Wv.T @ WoWih                                   # [E, G]
    wih = Wih.T.copy()                                  # [E, G]
    bih = (b_ih + b_hh)[idx] + (bqkv[2 * E:3 * E] @ Wo.T + bo) @ Wih.T
    if flip:
        Mq = Mq[::-1, ::-1]
        W3 = W3[::-1, :]
        wih = wih[::-1, :]
    m = {
        "mq": c(Mq.astype(BF_NP)),
        "w3": c(W3.astype(BF_NP)),
        "wih": c(wih.astype(np.float32)),
        "bih": c(bih.reshape(1, -1).astype(np.float32)),
    }
    bq = bqkv[0:E]
    if np.any(bq):
        u = (Wk.T @ bq) / 32.0                          # column shift of scores
        xx = x[:, ::-1] if flip else x
        m["vecb"] = c((xx @ u).reshape(1, B).astype(BF_NP))
    return m


def _build_in_maps(inputs, Wqkv_f, bqkv_f, Wo_f, bo_f, W_ih_f, b_ih_f, b_hh_f,
                   Wqkv_b, bqkv_b, Wo_b, bo_b, W_ih_b, b_ih_b, b_hh_b):
    inputs = np.asarray(inputs, dtype=np.float32)
    x = np.ascontiguousarray(inputs[:, -1, :])           # [B, E]

    shared = {}
    for d, args in (("f", (np.asarray(Wqkv_f), np.asarray(bqkv_f),
                           np.asarray(Wo_f), np.asarray(bo_f),
                           np.asarray(W_ih_f), np.asarray(b_ih_f),
                           np.asarray(b_hh_f), False, True)),
                    ("b", (np.asarray(Wqkv_b), np.asarray(bqkv_b),
                           np.asarray(Wo_b), np.asarray(bo_b),
                           np.asarray(W_ih_b), np.asarray(b_ih_b),
                           np.asarray(b_hh_b), True, False))):
        for k, v in _prep_dir(x, *args).items():
            shared[f"{k}_{d}"] = v
    with_attn_bias = "vecb_f" in shared or "vecb_b" in shared
    if with_attn_bias:
        for d in ("f", "b"):
            if f"vecb_{d}" not in shared:
                shared[f"vecb_{d}"] = np.zeros((1, B), dtype=BF_NP)

    c = np.ascontiguousarray
    in_maps = []
    for ci in range(N_CORES):
        xs = x[ci * BS:(ci + 1) * BS]                    # [128, E]
        xtpack = xs.reshape(BS, NE, 128).transpose(2, 1, 0).reshape(128, E)
        xtp = np.concatenate([xtpack, xs], axis=0).astype(BF_NP)
        m = {"xtp": c(xtp), "xT32": c(xs.T.astype(np.float32))}
        m.update(shared)
        in_maps.append(m)
    return in_maps, with_attn_bias


def kernel(inputs, Wqkv_f, bqkv_f, Wo_f, bo_f, W_ih_f, b_ih_f, b_hh_f,
           Wqkv_b, bqkv_b, Wo_b, bo_b, W_ih_b, b_ih_b, b_hh_b):
    in_maps, with_attn_bias = _build_in_maps(
        inputs, Wqkv_f, bqkv_f, Wo_f, bo_f, W_ih_f, b_ih_f, b_hh_f,
        Wqkv_b, bqkv_b, Wo_b, bo_b, W_ih_b, b_ih_b, b_hh_b)
    nc = _get_nc(with_attn_bias)
    res = run_bass_kernel_spmd(nc, in_maps, core_ids=list(range(N_CORES)))
    out = np.concatenate([res.results[ci]["out"] for ci in range(N_CORES)],
                         axis=0)
    return out.astype(np.float32)


# revision 16
# speedup vs baseline: 2.0895x; 1.0449x over previous
"""Trainium2 Bass kernel for nn_AttentionEnhancedBiLSTM (8 NeuronCores, SPMD).

Math (from the reference), per direction:
    x  = inputs[:, -1, :]                       # [B=1024, E=1024]
    scores = (x Wq^T)(x Wk^T)^T / 32
    af = softmax(scores) (x Wv^T) Wo^T + bo
    gates = (af + x) W_ih^T + b;  c = sig(i)*tanh(g);  h = sig(o)*tanh(c)
    out = concat([h_f, c_b], -1)   (backward direction uses xr = x[:, ::-1])

Factorization used here (exact in real arithmetic):
    scores = x Mq x^T,            Mq  = Wq^T Wk / 32
    gates  = rinv . (p @ x_all) @ W3 + x @ wih + bih_eff
        W3  = Wv^T Wo^T W_ih'^T   (W_ih' = used gate rows: fwd i,g,o; bwd i,g)
        wih = W_ih'^T
        bih_eff = (b_ih + b_hh)' + (bv Wo^T + bo) W_ih'^T   (softmax rows sum
        to 1, so bv/bo fold exactly; bk shifts scores per-row -> softmax
        invariant -> dropped; bq shifts per-column -> host vector vecb)
    The backward x-flip folds into the weights: Mq_b[::-1,::-1],
    W3_b[::-1,:], wih_b[::-1,:].  The device therefore needs only ONE
    AllGather of the raw x shard (transposed-pack + natural, bf16), shared by
    both directions, triggered at t=0 directly from the input DRAM tensor.

Precision: all attention matmuls bf16 (tested 1.5e-3 end-to-end rel err vs
2e-2 budget); the x @ wih gates branch stays float32r (bf16 there degrades to
1.2e-2).  Softmax runs without max-subtraction: |scores| <= ~6 so exp stays
comfortably inside f32/bf16 range.

Sharding: batch-sharded 8 ways (128 rows/core); weights replicated.
"""

import numpy as np
import ml_dtypes

import concourse.bass as bass
import concourse.mybir as mybir
import concourse.tile as tile
from concourse import bacc
from concourse.bass_utils import run_bass_kernel_spmd
from concourse.masks import make_identity

N_CORES = 8
B, T, E, H = 1024, 128, 1024, 512
BS = B // N_CORES          # 128 batch rows per core
NE = E // 128              # 8 e-chunks
GF = 3 * H                 # fwd gates i,g,o
GB = 2 * H                 # bwd gates i,g
F32 = mybir.dt.float32
FR = mybir.dt.float32r
BF = mybir.dt.bfloat16
BF_NP = ml_dtypes.bfloat16


def _emit(tc, nc, sb, ps, ident, ones32, onesb, ext, bounce_in, bounce_out,
          out_sb, with_attn_bias):
    Exp = mybir.ActivationFunctionType.Exp
    Sig = mybir.ActivationFunctionType.Sigmoid
    Tanh = mybir.ActivationFunctionType.Tanh
    Copy = mybir.ActivationFunctionType.Copy
    dirs = ("f", "b")
    G = {"f": GF, "b": GB}

    def transpose_1024(src, name, copy_on_scalar):
        """[128, 1024] bf16 natural -> chunk-transposed [128, 1024] bf16."""
        out = sb.tile([128, E], BF, name=name, tag=name)
        for half in range(2):
            tp = ps.tile([128, 512], BF, name=f"tp_{name}_{half}", tag="tp")
            for i in range(4):
                j = half * 4 + i
                nc.tensor.transpose(tp[:, i * 128:(i + 1) * 128],
                                    src[:, j * 128:(j + 1) * 128], ident[:])
            dst = out[:, half * 512:(half + 1) * 512]
            if copy_on_scalar:
                nc.scalar.activation(dst, tp[:], Copy)
            else:
                nc.vector.tensor_copy(dst, tp[:])
        return out

    # ---- t=0: AllGather of the raw x shard, split into two collectives so
    # the transposed half (needed first, by scores) lands ~15us earlier.
    # (collectives can't read IO tensors directly -> bounce through local DRAM)
    bi_t, bi_n = bounce_in
    bo_t, bo_n = bounce_out
    nc.sync.dma_start(bi_t.opt(), ext["xtp"][0:BS, :])
    nc.sync.dma_start(bi_n.opt(), ext["xtp"][BS:2 * BS, :])
    for bi, bo in ((bi_t, bo_t), (bi_n, bo_n)):
        nc.gpsimd.collective_compute(
            "AllGather",
            mybir.AluOpType.bypass,
            replica_groups=[list(range(N_CORES))],
            ins=[bi.opt()],
            outs=[bo.opt()],
        )

    # ---- w3 (whole matrices on the idle gpsimd queue; land during the AG) --
    w3 = {}
    for d in ("f", "b"):
        g = GF if d == "f" else GB
        w3_sb = sb.tile([128, NE * g], BF, name=f"w3_{d}", tag=f"w3_{d}")
        nc.gpsimd.dma_start(w3_sb[:], ext[f"w3_{d}"].rearrange(
            "(n p) m -> p n m", p=128))
        w3[d] = w3_sb.rearrange("p (n m) -> p n m", n=NE)

    # ---- local loads (sync DMA queue) ----
    xt = sb.tile([128, E], BF, name="xt", tag="xt")
    nc.sync.dma_start(xt[:], ext["xtp"][0:BS, :])
    xT32 = sb.tile([128, E], FR, name="xT32", tag="xT32")
    nc.sync.dma_start(xT32[:], ext["xT32"].rearrange("(n p) m -> p n m",
                                                     p=128))

    # ---- pre-AG: xM = x @ Mq, then transpose (both dirs) ------------------
    xmT = {}
    for d in dirs:
        xm_ps = ps.tile([128, E], F32, name=f"xm_{d}", tag="mm")
        for ec in range(NE):
            mqt = sb.tile([128, E], BF, name=f"mq_{d}_{ec}", tag="w")
            nc.scalar.dma_start(mqt[:], ext[f"mq_{d}"][ec * 128:(ec + 1) * 128, :])
            for n in range(2):
                nc.tensor.matmul(
                    xm_ps[:, n * 512:(n + 1) * 512],
                    xt[:, ec * 128:(ec + 1) * 128],
                    mqt[:, n * 512:(n + 1) * 512],
                    start=(ec == 0), stop=(ec == NE - 1),
                )
        xm_sb = sb.tile([128, E], BF, name=f"xmsb_{d}", tag=f"xmsb_{d}")
        for n in range(2):
            nc.vector.tensor_copy(xm_sb[:, n * 512:(n + 1) * 512],
                                  xm_ps[:, n * 512:(n + 1) * 512])
        xmT[d] = transpose_1024(xm_sb, f"xmT_{d}", copy_on_scalar=False)

    # ---- pre-AG: gx = x @ wih + bih  (f32r branch, both dirs) -------------
    # wih loads as one whole-matrix DMA per direction: chunked ring slots
    # create DMA-trigger -> PE-watermark convoys across the AG boundary.
    gx = {}
    for d in dirs:
        g = G[d]
        gx_ps = ps.tile([128, g], F32, name=f"gxps_{d}", tag="mm")
        for q in range(4):                     # quarter-matrix wih slices
            wt = sb.tile([128, 2, g], FR, name=f"wih_{d}_{q}", tag="w32")
            nc.sync.dma_start(wt[:], ext[f"wih_{d}"].rearrange(
                "(n p) m -> p n m", p=128)[:, 2 * q:2 * q + 2, :])
            for sub in range(2):
                ec = 2 * q + sub
                for n in range(g // 512):
                    nc.tensor.matmul(
                        gx_ps[:, n * 512:(n + 1) * 512],
                        xT32[:, ec * 128:(ec + 1) * 128],
                        wt[:, sub, n * 512:(n + 1) * 512],
                        start=(ec == 0), stop=False,
                    )
        bih = sb.tile([1, g], FR, name=f"bih_{d}", tag="bias")
        nc.sync.dma_start(bih[:], ext[f"bih_{d}"][:])
        for n in range(g // 512):
            nc.tensor.matmul(
                gx_ps[:, n * 512:(n + 1) * 512],
                ones32[0:1, :],
                bih[0:1, n * 512:(n + 1) * 512],
                start=False, stop=(n == g // 512 - 1),
            )
        gx_sb = sb.tile([128, g], F32, name=f"gx_{d}", tag=f"gx_{d}")
        for n in range(g // 512):
            nc.vector.tensor_copy(gx_sb[:, n * 512:(n + 1) * 512],
                                  gx_ps[:, n * 512:(n + 1) * 512])
        gx[d] = gx_sb

    # ---- post-AG: gathered x in both layouts (reads split across queues) --
    xTf = sb.tile([128, N_CORES * E], BF, name="xTf", tag="xTf")
    for g_ in range(N_CORES):
        eng = nc.gpsimd if g_ < 4 else nc.scalar
        eng.dma_start(xTf[:, g_ * E:(g_ + 1) * E], bo_t[g_, :, :])
    xnat = sb.tile([128, N_CORES * E], BF, name="xnat", tag="xnat")
    for g_ in range(N_CORES):
        eng = nc.sync if g_ < 4 else nc.gpsimd
        eng.dma_start(xnat[:, g_ * E:(g_ + 1) * E], bo_n[g_, :, :])
    xTf4 = xTf.rearrange("p (g x) -> p g x", g=N_CORES)

    # ---- scores for both dirs (fills the softmax_f PE bubble) -------------
    sc_ps = {}
    for d in dirs:
        acc = ps.tile([128, B], F32, name=f"sc_{d}", tag="mm")
        for jc in range(NE):
            for n in range(B // 512):
                nc.tensor.matmul(
                    acc[:, n * 512:(n + 1) * 512],
                    xmT[d][:, jc * 128:(jc + 1) * 128],
                    xTf4[:, 4 * n:4 * (n + 1), jc * 128:(jc + 1) * 128],
                    start=(jc == 0),
                    stop=(jc == NE - 1 and not with_attn_bias),
                )
        if with_attn_bias:
            vb = sb.tile([1, B], BF, name=f"vecb_{d}", tag="bias")
            nc.sync.dma_start(vb[:], ext[f"vecb_{d}"][:])
            for n in range(B // 512):
                nc.tensor.matmul(
                    acc[:, n * 512:(n + 1) * 512],
                    onesb[0:1, :],
                    vb[0:1, n * 512:(n + 1) * 512],
                    start=False, stop=(n == B // 512 - 1),
                )
        sc_ps[d] = acc

    # ---- softmax (no max-subtraction; |scores| <= ~6) + per-dir tail ------
    # Emission order is chosen so each engine's in-order stream never convoys:
    # scalar = exp_f, exp_b, ax copies, activations; DVE = recip/scale,
    # transpose copies, adds, muls; PE = pT_f, ax_f, axT_f, gp_f, pT_b, ...
    def softmax_exp(d):
        rowsum = sb.tile([128, 1], F32, name=f"rowsum_{d}", tag="stat")
        p_sb = sb.tile([128, B], BF, name=f"p_{d}", tag=f"p_{d}")
        nc.scalar.activation(p_sb[:], sc_ps[d][:], Exp, accum_out=rowsum[:])
        return p_sb, rowsum

    def softmax_scale(d, p_sb, rowsum):
        rinv = sb.tile([128, 1], F32, name=f"rinv_{d}", tag="stat")
        nc.vector.reciprocal(rinv[:], rowsum[:])
        pn_sb = sb.tile([128, B], BF, name=f"pn_{d}", tag=f"pn_{d}")
        for n in range(2):
            nc.vector.tensor_scalar_mul(pn_sb[:, n * 512:(n + 1) * 512],
                                        p_sb[:, n * 512:(n + 1) * 512],
                                        rinv[:])
        return pn_sb

    def dir_tail(d, pn_sb):
        g = G[d]
        pT = transpose_1024(pn_sb, f"pT_{d}", copy_on_scalar=False)
        ax_ps = ps.tile([128, E], F32, name=f"ax_{d}", tag="mm")
        for bc in range(NE):
            for n in range(2):
                nc.tensor.matmul(
                    ax_ps[:, n * 512:(n + 1) * 512],
                    pT[:, bc * 128:(bc + 1) * 128],
                    xnat[:, bc * E + n * 512: bc * E + (n + 1) * 512],
                    start=(bc == 0), stop=(bc == NE - 1),
                )
        ax_sb = sb.tile([128, E], BF, name=f"axsb_{d}", tag=f"axsb_{d}")
        for n in range(2):
            nc.scalar.activation(ax_sb[:, n * 512:(n + 1) * 512],
                                 ax_ps[:, n * 512:(n + 1) * 512], Copy)
        axT = transpose_1024(ax_sb, f"axT_{d}", copy_on_scalar=False)

        gp_ps = ps.tile([128, g], F32, name=f"gp_{d}", tag="mm")
        for ec in range(NE):
            for n in range(g // 512):
                nc.tensor.matmul(
                    gp_ps[:, n * 512:(n + 1) * 512],
                    axT[:, ec * 128:(ec + 1) * 128],
                    w3[d][:, ec, n * 512:(n + 1) * 512],
                    start=(ec == 0), stop=(ec == NE - 1),
                )
        gt = sb.tile([128, g], F32, name=f"gt_{d}", tag=f"gt_{d}")
        for n in range(g // 512):
            nc.vector.tensor_add(gt[:, n * 512:(n + 1) * 512],
                                 gp_ps[:, n * 512:(n + 1) * 512],
                                 gx[d][:, n * 512:(n + 1) * 512])

        si = sb.tile([128, H], F32, name=f"si_{d}", tag="gate")
        nc.scalar.activation(si[:], gt[:, 0:H], Sig)
        tg = sb.tile([128, H], F32, name=f"tg_{d}", tag="gate")
        nc.scalar.activation(tg[:], gt[:, H:2 * H], Tanh)
        if d == "f":
            cst = sb.tile([128, H], F32, name="c_f", tag="gate")
            nc.vector.tensor_mul(cst[:], si[:], tg[:])
            tc_ = sb.tile([128, H], F32, name="tc_f", tag="gate")
            nc.scalar.activation(tc_[:], cst[:], Tanh)
            so = sb.tile([128, H], F32, name="so_f", tag="gate")
            nc.scalar.activation(so[:], gt[:, 2 * H:3 * H], Sig)
            nc.vector.tensor_mul(out_sb[:, 0:H], so[:], tc_[:])
        else:
            nc.vector.tensor_mul(out_sb[:, H:2 * H], si[:], tg[:])

    p_f, rs_f = softmax_exp("f")
    pn_f = softmax_scale("f", p_f, rs_f)
    p_b, rs_b = softmax_exp("b")
    dir_tail("f", pn_f)
    pn_b = softmax_scale("b", p_b, rs_b)
    dir_tail("b", pn_b)


def build_nc(with_attn_bias=False):
    nc = bacc.Bacc("TRN2", target_bir_lowering=False, debug=False,
                   num_devices=N_CORES)

    def din(name, shape, dt):
        return nc.dram_tensor(name, shape, dt, kind="ExternalInput").ap()

    ext = {
        "xtp": din("xtp", [2 * BS, E], BF),
        "xT32": din("xT32", [E, BS], FR),
    }
    for d, g in (("f", GF), ("b", GB)):
        ext[f"mq_{d}"] = din(f"mq_{d}", [E, E], BF)
        ext[f"w3_{d}"] = din(f"w3_{d}", [E, g], BF)
        ext[f"wih_{d}"] = din(f"wih_{d}", [E, g], FR)
        ext[f"bih_{d}"] = din(f"bih_{d}", [1, g], FR)
        if with_attn_bias:
            ext[f"vecb_{d}"] = din(f"vecb_{d}", [1, B], BF)
    out_ext = nc.dram_tensor("out", [BS, 2 * H], F32, kind="ExternalOutput").ap()

    with tile.TileContext(nc) as tc:
        with (
            tc.tile_pool(name="sb", bufs=1) as sb_pool,
            tc.tile_pool(name="ps", bufs=1, space="PSUM") as ps_pool,
            tc.tile_pool(name="dram", bufs=1, space="DRAM") as dram_pool,
        ):
            class P:
                def __init__(self, pool, defaults):
                    self.pool, self.defaults = pool, defaults

                def tile(self, shape, dtype, name=None, tag=""):
                    bufs = self.defaults.get(tag, 1)
                    return self.pool.tile(shape, dtype, name=name, tag=tag,
                                          bufs=bufs)

            sb = P(sb_pool, {"w": 4, "w32": 2, "bias": 2, "gate": 6,
                             "stat": 4})
            ps = P(ps_pool, {"mm": 2, "tp": 2})

            identf = sb_pool.tile([128, 128], F32, name="identf", tag="identf")
            make_identity(nc, identf)
            ident = sb_pool.tile([128, 128], BF, name="ident", tag="ident")
            nc.vector.tensor_copy(ident[:], identf[:])
            onesf = sb_pool.tile([1, 128], F32, name="onesf", tag="onesf")
            nc.gpsimd.memset(onesf[:], 1.0)
            ones32 = sb_pool.tile([1, 128], FR, name="ones32", tag="ones32")
            nc.vector.tensor_copy(ones32[:], onesf[:])
            onesb = sb_pool.tile([1, 128], BF, name="onesb", tag="onesb")
            nc.vector.tensor_copy(onesb[:], onesf[:])

            bounce_in = (
                dram_pool.tile([BS, E], BF, name="bounce_in_t"),
                dram_pool.tile([BS, E], BF, name="bounce_in_n"),
            )
            bounce_out = (
                dram_pool.tile([N_CORES, BS, E], BF, name="bounce_t",
                               addr_space="Shared"),
                dram_pool.tile([N_CORES, BS, E], BF, name="bounce_n",
                               addr_space="Shared"),
            )
            out_sb = sb_pool.tile([BS, 2 * H], F32, name="out_sb", tag="out")

            _emit(tc, nc, sb, ps, ident, ones32, onesb, ext, bounce_in,
                  bounce_out, out_sb, with_attn_bias)

            nc.gpsimd.dma_start(out_ext[:], out_sb[:])

    nc.compile()
    return nc


_NC_CACHE = {}


def _get_nc(with_attn_bias=False):
    if with_attn_bias not in _NC_CACHE:
        _NC_CACHE[with_attn_bias] = build_nc(with_attn_bias)
    return _NC_CACHE[with_attn_bias]


def _prep_dir(x, Wqkv, bqkv, Wo, bo, W_ih, b_ih, b_hh, flip, want_h):
    """Host-side weight folds for one direction (shared across cores)."""
    c = np.ascontiguousarray
    Wq, Wk, Wv = Wqkv[0:E], Wqkv[E:2 * E], Wqkv[2 * E:3 * E]
    idx = np.r_[0:H, 2 * H:3 * H, 3 * H:4 * H] if want_h else np.r_[0:H, 2 * H:3 * H]
    Wih = W_ih[idx]                                     # [G, E]
    Mq = (Wq.T @ Wk) / 32.0                             # [E, E]
    WoWih = Wo.T @ Wih.T                                # [E, G]
    W3 = Wv.T @ WoWih                                   # [E, G]
    wih = Wih.T.copy()                                  # [E, G]
    bih = (b_ih + b_hh)[idx] + (bqkv[2 * E:3 * E] @ Wo.T + bo) @ Wih.T
    if flip:
        Mq = Mq[::-1, ::-1]
        W3 = W3[::-1, :]
        wih = wih[::-1, :]
    m = {
        "mq": c(Mq.astype(BF_NP)),
        "w3": c(W3.astype(BF_NP)),
        "wih": c(wih.astype(np.float32)),
        "bih": c(bih.reshape(1, -1).astype(np.float32)),
    }
    bq = bqkv[0:E]
    if np.any(bq):
        u = (Wk.T @ bq) / 32.0                          # column shift of scores
        xx = x[:, ::-1] if flip else x
        m["vecb"] = c((xx @ u).reshape(1, B).astype(BF_NP))
    return m


def _build_in_maps(inputs, Wqkv_f, bqkv_f, Wo_f, bo_f, W_ih_f, b_ih_f, b_hh_f,
                   Wqkv_b, bqkv_b, Wo_b, bo_b, W_ih_b, b_ih_b, b_hh_b):
    inputs = np.asarray(inputs, dtype=np.float32)
    x = np.ascontiguousarray(inputs[:, -1, :])           # [B, E]

    shared = {}
    for d, args in (("f", (np.asarray(Wqkv_f), np.asarray(bqkv_f),
                           np.asarray(Wo_f), np.asarray(bo_f),
                           np.asarray(W_ih_f), np.asarray(b_ih_f),
                           np.asarray(b_hh_f), False, True)),
                    ("b", (np.asarray(Wqkv_b), np.asarray(bqkv_b),
                           np.asarray(Wo_b), np.asarray(bo_b),
                           np.asarray(W_ih_b), np.asarray(b_ih_b),
                           np.asarray(b_hh_b), True, False))):
        for k, v in _prep_dir(x, *args).items():
            shared[f"{k}_{d}"] = v
    with_attn_bias = "vecb_f" in shared or "vecb_b" in shared
    if with_attn_bias:
        for d in ("f", "b"):
            if f"vecb_{d}" not in shared:
                shared[f"vecb_{d}"] = np.zeros((1, B), dtype=BF_NP)

    c = np.ascontiguousarray
    in_maps = []
    for ci in range(N_CORES):
        xs = x[ci * BS:(ci + 1) * BS]                    # [128, E]
        xtpack = xs.reshape(BS, NE, 128).transpose(2, 1, 0).reshape(128, E)
        xtp = np.concatenate([xtpack, xs], axis=0).astype(BF_NP)
        m = {"xtp": c(xtp), "xT32": c(xs.T.astype(np.float32))}
        m.update(shared)
        in_maps.append(m)
    return in_maps, with_attn_bias


def kernel(inputs, Wqkv_f, bqkv_f, Wo_f, bo_f, W_ih_f, b_ih_f, b_hh_f,
           Wqkv_b, bqkv_b, Wo_b, bo_b, W_ih_b, b_ih_b, b_hh_b):
    in_maps, with_attn_bias = _build_in_maps(
        inputs, Wqkv_f, bqkv_f, Wo_f, bo_f, W_ih_f, b_ih_f, b_hh_f,
        Wqkv_b, bqkv_b, Wo_b, bo_b, W_ih_b, b_ih_b, b_hh_b)
    nc = _get_nc(with_attn_bias)
    res = run_bass_kernel_spmd(nc, in_maps, core_ids=list(range(N_CORES)))
    out = np.concatenate([res.results[ci]["out"] for ci in range(N_CORES)],
                         axis=0)
    return out.astype(np.float32)


# revision 23
# speedup vs baseline: 2.4002x; 1.1487x over previous
"""Trainium2 Bass kernel for nn_AttentionEnhancedBiLSTM (8 NeuronCores, SPMD).

Math (from the reference), per direction:
    x  = inputs[:, -1, :]                       # [B=1024, E=1024]
    scores = (x Wq^T)(x Wk^T)^T / 32
    af = softmax(scores) (x Wv^T) Wo^T + bo
    gates = (af + x) W_ih^T + b;  c = sig(i)*tanh(g);  h = sig(o)*tanh(c)
    out = concat([h_f, c_b], -1)   (backward direction uses xr = x[:, ::-1])

Factorization used here (exact in real arithmetic):
    scores = x Mq x^T,            Mq  = Wq^T Wk / 32
    gates  = rinv . (p @ x_all) @ W3 + x @ wih + bih_eff
        W3  = Wv^T Wo^T W_ih'^T   (W_ih' = used gate rows: fwd i,g,o; bwd i,g)
        wih = W_ih'^T
        bih_eff = (b_ih + b_hh)' + (bv Wo^T + bo) W_ih'^T   (softmax rows sum
        to 1, so bv/bo fold exactly; bk shifts scores per-row -> softmax
        invariant -> dropped; bq shifts per-column -> host vector vecb)
    The backward x-flip folds into the weights: Mq_b[::-1,::-1],
    W3_b[::-1,:], wih_b[::-1,:].  The device therefore needs only ONE
    AllGather of the raw x shard (transposed-pack + natural, bf16), shared by
    both directions, triggered at t=0 directly from the input DRAM tensor.

Precision: all attention matmuls bf16 (tested 1.5e-3 end-to-end rel err vs
2e-2 budget); the x @ wih gates branch stays float32r (bf16 there degrades to
1.2e-2).  Softmax runs without max-subtraction: |scores| <= ~6 so exp stays
comfortably inside f32/bf16 range.

Sharding: batch-sharded 8 ways (128 rows/core); weights replicated.
"""

import numpy as np
import ml_dtypes

import concourse.bass as bass
import concourse.mybir as mybir
import concourse.tile as tile
from concourse import bacc
from concourse.bass_utils import run_bass_kernel_spmd
from concourse.masks import make_identity

N_CORES = 8
B, T, E, H = 1024, 128, 1024, 512
BS = B // N_CORES          # 128 batch rows per core
NE = E // 128              # 8 e-chunks
GF = 3 * H                 # fwd gates i,g,o
GB = 2 * H                 # bwd gates i,g
F32 = mybir.dt.float32
FR = mybir.dt.float32r
BF = mybir.dt.bfloat16
BF_NP = ml_dtypes.bfloat16


def _emit(tc, nc, sb, ps, ident, ones32, onesb, ext, bounce_in, bounce_out,
          out_sb, with_attn_bias):
    Exp = mybir.ActivationFunctionType.Exp
    Sig = mybir.ActivationFunctionType.Sigmoid
    Tanh = mybir.ActivationFunctionType.Tanh
    Copy = mybir.ActivationFunctionType.Copy
    dirs = ("f", "b")
    G = {"f": GF, "b": GB}

    def transpose_1024(src, name, copy_on_scalar):
        """[128, 1024] bf16 natural -> chunk-transposed [128, 1024] bf16."""
        out = sb.tile([128, E], BF, name=name, tag=name)
        for half in range(2):
            tp = ps.tile([128, 512], BF, name=f"tp_{name}_{half}", tag="tp")
            for i in range(4):
                j = half * 4 + i
                nc.tensor.transpose(tp[:, i * 128:(i + 1) * 128],
                                    src[:, j * 128:(j + 1) * 128], ident[:])
            dst = out[:, half * 512:(half + 1) * 512]
            if copy_on_scalar:
                nc.scalar.activation(dst, tp[:], Copy)
            else:
                nc.vector.tensor_copy(dst, tp[:])
        return out

    # ---- t=0: ONE AllGather of the transposed-pack x shard (2MB out); the
    # natural layout is rebuilt on-device by PE transposes (cheaper than a
    # second collective's ~25us protocol cost).
    # (collectives can't read IO tensors directly -> bounce through local DRAM)
    nc.sync.dma_start(bounce_in.opt(), ext["xtp"][:])
    nc.gpsimd.collective_compute(
        "AllGather",
        mybir.AluOpType.bypass,
        replica_groups=[list(range(N_CORES))],
        ins=[bounce_in.opt()],
        outs=[bounce_out.opt()],
    )

    # ---- w3 (whole matrices on the idle gpsimd queue; land during the AG) --
    w3 = {}
    for d in ("f", "b"):
        g = GF if d == "f" else GB
        w3_sb = sb.tile([128, NE * g], BF, name=f"w3_{d}", tag=f"w3_{d}")
        nc.gpsimd.dma_start(w3_sb[:], ext[f"w3_{d}"].rearrange(
            "(n p) m -> p n m", p=128))
        w3[d] = w3_sb.rearrange("p (n m) -> p n m", n=NE)

    # ---- local loads (sync DMA queue) ----
    xt = sb.tile([128, E], BF, name="xt", tag="xt")
    nc.sync.dma_start(xt[:], ext["xtp"][:])
    xT32 = sb.tile([128, E], FR, name="xT32", tag="xT32")
    nc.sync.dma_start(xT32[:], ext["xT32"].rearrange("(n p) m -> p n m",
                                                     p=128))

    # ---- pre-AG: xM = x @ Mq, then transpose (both dirs) ------------------
    xmT = {}
    for d in dirs:
        xm_ps = ps.tile([128, E], F32, name=f"xm_{d}", tag="mm")
        for ec in range(NE):
            mqt = sb.tile([128, E], BF, name=f"mq_{d}_{ec}", tag="w")
            nc.scalar.dma_start(mqt[:], ext[f"mq_{d}"][ec * 128:(ec + 1) * 128, :])
            for n in range(2):
                nc.tensor.matmul(
                    xm_ps[:, n * 512:(n + 1) * 512],
                    xt[:, ec * 128:(ec + 1) * 128],
                    mqt[:, n * 512:(n + 1) * 512],
                    start=(ec == 0), stop=(ec == NE - 1),
                )
        xm_sb = sb.tile([128, E], BF, name=f"xmsb_{d}", tag=f"xmsb_{d}")
        for n in range(2):
            nc.vector.tensor_copy(xm_sb[:, n * 512:(n + 1) * 512],
                                  xm_ps[:, n * 512:(n + 1) * 512])
        xmT[d] = transpose_1024(xm_sb, f"xmT_{d}", copy_on_scalar=False)

    # ---- pre-AG: gx = x @ wih + bih  (f32r branch, both dirs) -------------
    # wih loads as one whole-matrix DMA per direction: chunked ring slots
    # create DMA-trigger -> PE-watermark convoys across the AG boundary.
    gx = {}
    for d in dirs:
        g = G[d]
        gx_ps = ps.tile([128, g], F32, name=f"gxps_{d}", tag="mm")
        for q in range(4):                     # quarter-matrix wih slices
            wt = sb.tile([128, 2, g], FR, name=f"wih_{d}_{q}", tag="w32")
            nc.sync.dma_start(wt[:], ext[f"wih_{d}"].rearrange(
                "(n p) m -> p n m", p=128)[:, 2 * q:2 * q + 2, :])
            for sub in range(2):
                ec = 2 * q + sub
                for n in range(g // 512):
                    nc.tensor.matmul(
                        gx_ps[:, n * 512:(n + 1) * 512],
                        xT32[:, ec * 128:(ec + 1) * 128],
                        wt[:, sub, n * 512:(n + 1) * 512],
                        start=(ec == 0), stop=False,
                    )
        bih = sb.tile([1, g], FR, name=f"bih_{d}", tag="bias")
        nc.sync.dma_start(bih[:], ext[f"bih_{d}"][:])
        for n in range(g // 512):
            nc.tensor.matmul(
                gx_ps[:, n * 512:(n + 1) * 512],
                ones32[0:1, :],
                bih[0:1, n * 512:(n + 1) * 512],
                start=False, stop=(n == g // 512 - 1),
            )
        gx_sb = sb.tile([128, g], F32, name=f"gx_{d}", tag=f"gx_{d}")
        for n in range(g // 512):
            nc.vector.tensor_copy(gx_sb[:, n * 512:(n + 1) * 512],
                                  gx_ps[:, n * 512:(n + 1) * 512])
        gx[d] = gx_sb

    # ---- post-AG: gathered xT (reads split across queues) -----------------
    xTf = sb.tile([128, N_CORES * E], BF, name="xTf", tag="xTf")
    for g_ in range(N_CORES):
        eng = nc.gpsimd if g_ < 4 else nc.scalar
        eng.dma_start(xTf[:, g_ * E:(g_ + 1) * E], bounce_out[g_, :, :])
    xTf4 = xTf.rearrange("p (g x) -> p g x", g=N_CORES)
    xnat = sb.tile([128, N_CORES * E], BF, name="xnat", tag="xnat")

    # ---- scores for both dirs (fills the softmax_f PE bubble) -------------
    sc_ps = {}
    for d in dirs:
        acc = ps.tile([128, B], F32, name=f"sc_{d}", tag="mm")
        for jc in range(NE):
            for n in range(B // 512):
                nc.tensor.matmul(
                    acc[:, n * 512:(n + 1) * 512],
                    xmT[d][:, jc * 128:(jc + 1) * 128],
                    xTf4[:, 4 * n:4 * (n + 1), jc * 128:(jc + 1) * 128],
                    start=(jc == 0),
                    stop=(jc == NE - 1 and not with_attn_bias),
                )
        if with_attn_bias:
            vb = sb.tile([1, B], BF, name=f"vecb_{d}", tag="bias")
            nc.sync.dma_start(vb[:], ext[f"vecb_{d}"][:])
            for n in range(B // 512):
                nc.tensor.matmul(
                    acc[:, n * 512:(n + 1) * 512],
                    onesb[0:1, :],
                    vb[0:1, n * 512:(n + 1) * 512],
                    start=False, stop=(n == B // 512 - 1),
                )
        sc_ps[d] = acc

    # ---- softmax (no max-subtraction; |scores| <= ~6) + per-dir tail ------
    # Emission order is chosen so each engine's in-order stream never convoys:
    # scalar = exp_f, exp_b, ax copies, activations; DVE = recip/scale,
    # transpose copies, adds, muls; PE = pT_f, ax_f, axT_f, gp_f, pT_b, ...
    def softmax_exp(d):
        rowsum = sb.tile([128, 1], F32, name=f"rowsum_{d}", tag="stat")
        p_sb = sb.tile([128, B], BF, name=f"p_{d}", tag=f"p_{d}")
        nc.scalar.activation(p_sb[:], sc_ps[d][:], Exp, accum_out=rowsum[:])
        return p_sb, rowsum

    def softmax_scale(d, p_sb, rowsum):
        rinv = sb.tile([128, 1], F32, name=f"rinv_{d}", tag="stat")
        nc.vector.reciprocal(rinv[:], rowsum[:])
        pn_sb = sb.tile([128, B], BF, name=f"pn_{d}", tag=f"pn_{d}")
        for n in range(2):
            nc.vector.tensor_scalar_mul(pn_sb[:, n * 512:(n + 1) * 512],
                                        p_sb[:, n * 512:(n + 1) * 512],
                                        rinv[:])
        return pn_sb

    def dir_tail(d, pn_sb):
        g = G[d]
        pT = transpose_1024(pn_sb, f"pT_{d}", copy_on_scalar=False)
        ax_ps = ps.tile([128, E], F32, name=f"ax_{d}", tag="mm")
        for bc in range(NE):
            for n in range(2):
                nc.tensor.matmul(
                    ax_ps[:, n * 512:(n + 1) * 512],
                    pT[:, bc * 128:(bc + 1) * 128],
                    xnat[:, bc * E + n * 512: bc * E + (n + 1) * 512],
                    start=(bc == 0), stop=(bc == NE - 1),
                )
        ax_sb = sb.tile([128, E], BF, name=f"axsb_{d}", tag=f"axsb_{d}")
        for n in range(2):
            nc.scalar.activation(ax_sb[:, n * 512:(n + 1) * 512],
                                 ax_ps[:, n * 512:(n + 1) * 512], Copy)
        axT = transpose_1024(ax_sb, f"axT_{d}", copy_on_scalar=False)

        gp_ps = ps.tile([128, g], F32, name=f"gp_{d}", tag="mm")
        for ec in range(NE):
            for n in range(g // 512):
                nc.tensor.matmul(
                    gp_ps[:, n * 512:(n + 1) * 512],
                    axT[:, ec * 128:(ec + 1) * 128],
                    w3[d][:, ec, n * 512:(n + 1) * 512],
                    start=(ec == 0), stop=(ec == NE - 1),
                )
        gt = sb.tile([128, g], F32, name=f"gt_{d}", tag=f"gt_{d}")
        for n in range(g // 512):
            nc.vector.tensor_add(gt[:, n * 512:(n + 1) * 512],
                                 gp_ps[:, n * 512:(n + 1) * 512],
                                 gx[d][:, n * 512:(n + 1) * 512])

        si = sb.tile([128, H], F32, name=f"si_{d}", tag="gate")
        nc.scalar.activation(si[:], gt[:, 0:H], Sig)
        tg = sb.tile([128, H], F32, name=f"tg_{d}", tag="gate")
        nc.scalar.activation(tg[:], gt[:, H:2 * H], Tanh)
        if d == "f":
            cst = sb.tile([128, H], F32, name="c_f", tag="gate")
            nc.vector.tensor_mul(cst[:], si[:], tg[:])
            tc_ = sb.tile([128, H], F32, name="tc_f", tag="gate")
            nc.scalar.activation(tc_[:], cst[:], Tanh)
            so = sb.tile([128, H], F32, name="so_f", tag="gate")
            nc.scalar.activation(so[:], gt[:, 2 * H:3 * H], Sig)
            nc.vector.tensor_mul(out_sb[:, 0:H], so[:], tc_[:])
        else:
            nc.vector.tensor_mul(out_sb[:, H:2 * H], si[:], tg[:])

    p_f, rs_f = softmax_exp("f")
    pn_f = softmax_scale("f", p_f, rs_f)

    # rebuild x-natural from gathered xT: 64 PE transposes, copies on DVE.
    # PE order puts these after both scores blocks, before the dir-f tail.
    for g_ in range(N_CORES):
        for n in range(2):
            tp = ps.tile([128, 512], BF, name=f"xn_{g_}_{n}", tag="tp")
            for i in range(4):
                jc = 4 * n + i
                nc.tensor.transpose(
                    tp[:, i * 128:(i + 1) * 128],
                    xTf[:, g_ * E + jc * 128: g_ * E + (jc + 1) * 128],
                    ident[:])
            nc.vector.tensor_copy(
                xnat[:, g_ * E + n * 512: g_ * E + (n + 1) * 512], tp[:])

    p_b, rs_b = softmax_exp("b")
    dir_tail("f", pn_f)
    pn_b = softmax_scale("b", p_b, rs_b)
    dir_tail("b", pn_b)


def build_nc(with_attn_bias=False):
    nc = bacc.Bacc("TRN2", target_bir_lowering=False, debug=False,
                   num_devices=N_CORES)

    def din(name, shape, dt):
        return nc.dram_tensor(name, shape, dt, kind="ExternalInput").ap()

    ext = {
        "xtp": din("xtp", [BS, E], BF),
        "xT32": din("xT32", [E, BS], FR),
    }
    for d, g in (("f", GF), ("b", GB)):
        ext[f"mq_{d}"] = din(f"mq_{d}", [E, E], BF)
        ext[f"w3_{d}"] = din(f"w3_{d}", [E, g], BF)
        ext[f"wih_{d}"] = din(f"wih_{d}", [E, g], FR)
        ext[f"bih_{d}"] = din(f"bih_{d}", [1, g], FR)
        if with_attn_bias:
            ext[f"vecb_{d}"] = din(f"vecb_{d}", [1, B], BF)
    out_ext = nc.dram_tensor("out", [BS, 2 * H], F32, kind="ExternalOutput").ap()

    with tile.TileContext(nc) as tc:
        with (
            tc.tile_pool(name="sb", bufs=1) as sb_pool,
            tc.tile_pool(name="ps", bufs=1, space="PSUM") as ps_pool,
            tc.tile_pool(name="dram", bufs=1, space="DRAM") as dram_pool,
        ):
            class P:
                def __init__(self, pool, defaults):
                    self.pool, self.defaults = pool, defaults

                def tile(self, shape, dtype, name=None, tag=""):
                    bufs = self.defaults.get(tag, 1)
                    return self.pool.tile(shape, dtype, name=name, tag=tag,
                                          bufs=bufs)

            sb = P(sb_pool, {"w": 4, "w32": 2, "bias": 2, "gate": 6,
                             "stat": 4})
            ps = P(ps_pool, {"mm": 2, "tp": 2})

            identf = sb_pool.tile([128, 128], F32, name="identf", tag="identf")
            make_identity(nc, identf)
            ident = sb_pool.tile([128, 128], BF, name="ident", tag="ident")
            nc.vector.tensor_copy(ident[:], identf[:])
            onesf = sb_pool.tile([1, 128], F32, name="onesf", tag="onesf")
            nc.gpsimd.memset(onesf[:], 1.0)
            ones32 = sb_pool.tile([1, 128], FR, name="ones32", tag="ones32")
            nc.vector.tensor_copy(ones32[:], onesf[:])
            onesb = sb_pool.tile([1, 128], BF, name="onesb", tag="onesb")
            nc.vector.tensor_copy(onesb[:], onesf[:])

            bounce_in = dram_pool.tile([BS, E], BF, name="bounce_in_t")
            bounce_out = dram_pool.tile([N_CORES, BS, E], BF, name="bounce_t",
                                        addr_space="Shared")
            out_sb = sb_pool.tile([BS, 2 * H], F32, name="out_sb", tag="out")

            _emit(tc, nc, sb, ps, ident, ones32, onesb, ext, bounce_in,
                  bounce_out, out_sb, with_attn_bias)

            nc.gpsimd.dma_start(out_ext[:], out_sb[:])

    nc.compile()
    return nc


_NC_CACHE = {}


def _get_nc(with_attn_bias=False):
    if with_attn_bias not in _NC_CACHE:
        _NC_CACHE[with_attn_bias] = build_nc(with_attn_bias)
    return _NC_CACHE[with_attn_bias]


def _prep_dir(x, Wqkv, bqkv, Wo, bo, W_ih, b_ih, b_hh, flip, want_h):
    """Host-side weight folds for one direction (shared across cores)."""
    c = np.ascontiguousarray
    Wq, Wk, Wv = Wqkv[0:E], Wqkv[E:2 * E], Wqkv[2 * E:3 * E]
    idx = np.r_[0:H, 2 * H:3 * H, 3 * H:4 * H] if want_h else np.r_[0:H, 2 * H:3 * H]
    Wih = W_ih[idx]                                     # [G, E]
    Mq = (Wq.T @ Wk) / 32.0                             # [E, E]
    WoWih = Wo.T @ Wih.T                                # [E, G]
    W3 = Wv.T @ WoWih                                   # [E, G]
    wih = Wih.T.copy()                                  # [E, G]
    bih = (b_ih + b_hh)[idx] + (bqkv[2 * E:3 * E] @ Wo.T + bo) @ Wih.T
    if flip:
        Mq = Mq[::-1, ::-1]
        W3 = W3[::-1, :]
        wih = wih[::-1, :]
    m = {
        "mq": c(Mq.astype(BF_NP)),
        "w3": c(W3.astype(BF_NP)),
        "wih": c(wih.astype(np.float32)),
        "bih": c(bih.reshape(1, -1).astype(np.float32)),
    }
    bq = bqkv[0:E]
    if np.any(bq):
        u = (Wk.T @ bq) / 32.0                          # column shift of scores
        xx = x[:, ::-1] if flip else x
        m["vecb"] = c((xx @ u).reshape(1, B).astype(BF_NP))
    return m


def _build_in_maps(inputs, Wqkv_f, bqkv_f, Wo_f, bo_f, W_ih_f, b_ih_f, b_hh_f,
                   Wqkv_b, bqkv_b, Wo_b, bo_b, W_ih_b, b_ih_b, b_hh_b):
    inputs = np.asarray(inputs, dtype=np.float32)
    x = np.ascontiguousarray(inputs[:, -1, :])           # [B, E]

    shared = {}
    for d, args in (("f", (np.asarray(Wqkv_f), np.asarray(bqkv_f),
                           np.asarray(Wo_f), np.asarray(bo_f),
                           np.asarray(W_ih_f), np.asarray(b_ih_f),
                           np.asarray(b_hh_f), False, True)),
                    ("b", (np.asarray(Wqkv_b), np.asarray(bqkv_b),
                           np.asarray(Wo_b), np.asarray(bo_b),
                           np.asarray(W_ih_b), np.asarray(b_ih_b),
                           np.asarray(b_hh_b), True, False))):
        for k, v in _prep_dir(x, *args).items():
            shared[f"{k}_{d}"] = v
    with_attn_bias = "vecb_f" in shared or "vecb_b" in shared
    if with_attn_bias:
        for d in ("f", "b"):
            if f"vecb_{d}" not in shared:
                shared[f"vecb_{d}"] = np.zeros((1, B), dtype=BF_NP)

    c = np.ascontiguousarray
    in_maps = []
    for ci in range(N_CORES):
        xs = x[ci * BS:(ci + 1) * BS]                    # [128, E]
        xtpack = xs.reshape(BS, NE, 128).transpose(2, 1, 0).reshape(128, E)
        m = {"xtp": c(xtpack.astype(BF_NP)),
             "xT32": c(xs.T.astype(np.float32))}
        m.update(shared)
        in_maps.append(m)
    return in_maps, with_attn_bias


def kernel(inputs, Wqkv_f, bqkv_f, Wo_f, bo_f, W_ih_f, b_ih_f, b_hh_f,
           Wqkv_b, bqkv_b, Wo_b, bo_b, W_ih_b, b_ih_b, b_hh_b):
    in_maps, with_attn_bias = _build_in_maps(
        inputs, Wqkv_f, bqkv_f, Wo_f, bo_f, W_ih_f, b_ih_f, b_hh_f,
        Wqkv_b, bqkv_b, Wo_b, bo_b, W_ih_b, b_ih_b, b_hh_b)
    nc = _get_nc(with_attn_bias)
    res = run_bass_kernel_spmd(nc, in_maps, core_ids=list(range(N_CORES)))
    out = np.concatenate([res.results[ci]["out"] for ci in range(N_CORES)],
                         axis=0)
    return out.astype(np.float32)


# revision 26
# speedup vs baseline: 2.9619x; 1.2340x over previous
"""Trainium2 Bass kernel for nn_AttentionEnhancedBiLSTM (8 NeuronCores, SPMD).

Math (from the reference), per direction:
    x  = inputs[:, -1, :]                       # [B=1024, E=1024]
    scores = (x Wq^T)(x Wk^T)^T / 32
    af = softmax(scores) (x Wv^T) Wo^T + bo
    gates = (af + x) W_ih^T + b;  c = sig(i)*tanh(g);  h = sig(o)*tanh(c)
    out = concat([h_f, c_b], -1)   (backward direction uses xr = x[:, ::-1])

Factorization used here (exact in real arithmetic):
    scores = x Mq x^T,            Mq  = Wq^T Wk / 32
    gates  = rinv . (p @ x_all) @ W3 + x @ wih + bih_eff
        W3  = Wv^T Wo^T W_ih'^T   (W_ih' = used gate rows: fwd i,g,o; bwd i,g)
        wih = W_ih'^T
        bih_eff = (b_ih + b_hh)' + (bv Wo^T + bo) W_ih'^T   (softmax rows sum
        to 1, so bv/bo fold exactly; bk shifts scores per-row -> softmax
        invariant -> dropped; bq shifts per-column -> host vector vecb)
    The backward x-flip folds into the weights: Mq_b[::-1,::-1],
    W3_b[::-1,:], wih_b[::-1,:].

After this factorization the ONLY cross-core tensor is x itself -- an input.
So there is NO collective at all: every core receives the full x (in both
layouts, host-packed) as a replicated input, and computes its own 128 output
rows end-to-end.  q/k/v/af/lstm_in never materialize on device.

Precision: everything fp16 (10-bit mantissa; measured 1.7e-3 end-to-end rel
err vs the 2e-2 budget), accumulation in fp32 PSUM.  Softmax runs without
max-subtraction: |scores| <= ~6 so exp stays well inside fp16/fp32 range.

Sharding: batch-sharded 8 ways (128 rows/core); weights + x replicated.
"""

import numpy as np

import concourse.bass as bass
import concourse.mybir as mybir
import concourse.tile as tile
from concourse import bacc
from concourse.bass_utils import run_bass_kernel_spmd
from concourse.masks import make_identity

N_CORES = 8
B, T, E, H = 1024, 128, 1024, 512
BS = B // N_CORES          # 128 batch rows per core
NE = E // 128              # 8 e-chunks
GF = 3 * H                 # fwd gates i,g,o
GB = 2 * H                 # bwd gates i,g
F32 = mybir.dt.float32
FP = mybir.dt.float16
FP_NP = np.float16


def _emit(tc, nc, sb, ps, ext, out_sb, with_attn_bias):
    Exp = mybir.ActivationFunctionType.Exp
    Sig = mybir.ActivationFunctionType.Sigmoid
    Tanh = mybir.ActivationFunctionType.Tanh
    Copy = mybir.ActivationFunctionType.Copy
    dirs = ("f", "b")
    G = {"f": GF, "b": GB}

    # ---- gathered-x loads: full x is a replicated INPUT (no collective) ---
    xTf = sb.tile([128, N_CORES * E], FP, name="xTf", tag="xTf")
    for q in range(4):
        nc.gpsimd.dma_start(xTf[:, q * 2 * E:(q + 1) * 2 * E],
                            ext["xtp"][:, q * 2 * E:(q + 1) * 2 * E])
    xTf4 = xTf.rearrange("p (g x) -> p g x", g=N_CORES)

    # ---- local loads (sync DMA queue) ----
    xt = sb.tile([128, E], FP, name="xt", tag="xt")
    nc.sync.dma_start(xt[:], ext["xt"][:])
    xT16 = sb.tile([128, E], FP, name="xT16", tag="xT16")
    nc.sync.dma_start(xT16[:], ext["xT16"].rearrange("(n p) m -> p n m",
                                                     p=128))

    # ---- constants ----
    identf = sb.tile([128, 128], F32, name="identf", tag="identf")
    make_identity(nc, identf[:])
    ident = sb.tile([128, 128], FP, name="ident", tag="ident")
    nc.vector.tensor_copy(ident[:], identf[:])
    onesf = sb.tile([1, 128], F32, name="onesf", tag="onesf")
    nc.gpsimd.memset(onesf[:], 1.0)
    ones = sb.tile([1, 128], FP, name="ones", tag="ones")
    nc.vector.tensor_copy(ones[:], onesf[:])

    def transpose_1024(src, name):
        """[128, 1024] fp16 natural -> chunk-transposed [128, 1024] fp16."""
        out = sb.tile([128, E], FP, name=name, tag=name)
        for half in range(2):
            tp = ps.tile([128, 512], FP, name=f"tp_{name}_{half}", tag="tp")
            for i in range(4):
                j = half * 4 + i
                nc.tensor.transpose(tp[:, i * 128:(i + 1) * 128],
                                    src[:, j * 128:(j + 1) * 128], ident[:])
            nc.vector.tensor_copy(out[:, half * 512:(half + 1) * 512], tp[:])
        return out

    # ---- xnat: full x natural, host-packed (scalar queue, needed at ax) ---
    xnat = sb.tile([128, N_CORES * E], FP, name="xnat", tag="xnat")
    for q in range(4):
        nc.scalar.dma_start(xnat[:, q * 2 * E:(q + 1) * 2 * E],
                            ext["xnp"][:, q * 2 * E:(q + 1) * 2 * E])

    # ---- w3 (whole matrices, needed only at gp) ---------------------------
    w3 = {}
    for d in dirs:
        g = G[d]
        w3_sb = sb.tile([128, NE * g], FP, name=f"w3_{d}", tag=f"w3_{d}")
        eng = nc.gpsimd if d == "f" else nc.scalar
        eng.dma_start(w3_sb[:], ext[f"w3_{d}"].rearrange(
            "(n p) m -> p n m", p=128))
        w3[d] = w3_sb.rearrange("p (n m) -> p n m", n=NE)

    # ---- xM = x @ Mq, then transpose (both dirs) --------------------------
    xmT = {}
    for d in dirs:
        xm_ps = ps.tile([128, E], F32, name=f"xm_{d}", tag="mm")
        for ec in range(NE):
            mqt = sb.tile([128, E], FP, name=f"mq_{d}_{ec}", tag="w")
            nc.scalar.dma_start(mqt[:], ext[f"mq_{d}"][ec * 128:(ec + 1) * 128, :])
            for n in range(2):
                nc.tensor.matmul(
                    xm_ps[:, n * 512:(n + 1) * 512],
                    xt[:, ec * 128:(ec + 1) * 128],
                    mqt[:, n * 512:(n + 1) * 512],
                    start=(ec == 0), stop=(ec == NE - 1),
                )
        xm_sb = sb.tile([128, E], FP, name=f"xmsb_{d}", tag=f"xmsb_{d}")
        for n in range(2):
            nc.vector.tensor_copy(xm_sb[:, n * 512:(n + 1) * 512],
                                  xm_ps[:, n * 512:(n + 1) * 512])
        xmT[d] = transpose_1024(xm_sb, f"xmT_{d}")

    # ---- gx = x @ wih + bih  (both dirs) ----------------------------------
    gx = {}
    for d in dirs:
        g = G[d]
        gx_ps = ps.tile([128, g], F32, name=f"gxps_{d}", tag="mm")
        for q in range(4):                     # quarter-matrix wih slices
            wt = sb.tile([128, 2, g], FP, name=f"wih_{d}_{q}", tag="w32")
            weng = nc.sync if d == "f" else nc.gpsimd
            weng.dma_start(wt[:], ext[f"wih_{d}"].rearrange(
                "(n p) m -> p n m", p=128)[:, 2 * q:2 * q + 2, :])
            for sub in range(2):
                ec = 2 * q + sub
                for n in range(g // 512):
                    nc.tensor.matmul(
                        gx_ps[:, n * 512:(n + 1) * 512],
                        xT16[:, ec * 128:(ec + 1) * 128],
                        wt[:, sub, n * 512:(n + 1) * 512],
                        start=(ec == 0), stop=False,
                    )
        bih = sb.tile([1, g], FP, name=f"bih_{d}", tag="bias")
        (nc.sync if d == "f" else nc.gpsimd).dma_start(bih[:],
                                                       ext[f"bih_{d}"][:])
        for n in range(g // 512):
            nc.tensor.matmul(
                gx_ps[:, n * 512:(n + 1) * 512],
                ones[0:1, :],
                bih[0:1, n * 512:(n + 1) * 512],
                start=False, stop=(n == g // 512 - 1),
            )
        gx_sb = sb.tile([128, g], F32, name=f"gx_{d}", tag=f"gx_{d}")
        for n in range(g // 512):
            nc.vector.tensor_copy(gx_sb[:, n * 512:(n + 1) * 512],
                                  gx_ps[:, n * 512:(n + 1) * 512])
        gx[d] = gx_sb

    # ---- scores for both dirs ---------------------------------------------
    sc_ps = {}
    for d in dirs:
        acc = ps.tile([128, B], F32, name=f"sc_{d}", tag="mm")
        for jc in range(NE):
            for n in range(B // 512):
                nc.tensor.matmul(
                    acc[:, n * 512:(n + 1) * 512],
                    xmT[d][:, jc * 128:(jc + 1) * 128],
                    xTf4[:, 4 * n:4 * (n + 1), jc * 128:(jc + 1) * 128],
                    start=(jc == 0),
                    stop=(jc == NE - 1 and not with_attn_bias),
                )
        if with_attn_bias:
            vb = sb.tile([1, B], FP, name=f"vecb_{d}", tag="bias")
            nc.sync.dma_start(vb[:], ext[f"vecb_{d}"][:])
            for n in range(B // 512):
                nc.tensor.matmul(
                    acc[:, n * 512:(n + 1) * 512],
                    ones[0:1, :],
                    vb[0:1, n * 512:(n + 1) * 512],
                    start=False, stop=(n == B // 512 - 1),
                )
        sc_ps[d] = acc

    # ---- softmax (no max-subtraction; |scores| <= ~6) + per-dir tail ------
    def softmax_exp(d):
        rowsum = sb.tile([128, 1], F32, name=f"rowsum_{d}", tag="stat")
        p_sb = sb.tile([128, B], FP, name=f"p_{d}", tag=f"p_{d}")
        nc.scalar.activation(p_sb[:], sc_ps[d][:], Exp, accum_out=rowsum[:])
        return p_sb, rowsum

    def softmax_scale(d, p_sb, rowsum):
        rinv = sb.tile([128, 1], F32, name=f"rinv_{d}", tag="stat")
        nc.vector.reciprocal(rinv[:], rowsum[:])
        pn_sb = sb.tile([128, B], FP, name=f"pn_{d}", tag=f"pn_{d}")
        for n in range(2):
            nc.vector.tensor_scalar_mul(pn_sb[:, n * 512:(n + 1) * 512],
                                        p_sb[:, n * 512:(n + 1) * 512],
                                        rinv[:])
        return pn_sb

    def dir_tail(d, pn_sb):
        g = G[d]
        pT = transpose_1024(pn_sb, f"pT_{d}")
        ax_ps = ps.tile([128, E], F32, name=f"ax_{d}", tag="mm")
        for bc in range(NE):
            for n in range(2):
                nc.tensor.matmul(
                    ax_ps[:, n * 512:(n + 1) * 512],
                    pT[:, bc * 128:(bc + 1) * 128],
                    xnat[:, bc * E + n * 512: bc * E + (n + 1) * 512],
                    start=(bc == 0), stop=(bc == NE - 1),
                )
        ax_sb = sb.tile([128, E], FP, name=f"axsb_{d}", tag=f"axsb_{d}")
        for n in range(2):
            nc.scalar.activation(ax_sb[:, n * 512:(n + 1) * 512],
                                 ax_ps[:, n * 512:(n + 1) * 512], Copy)
        axT = transpose_1024(ax_sb, f"axT_{d}")

        gp_ps = ps.tile([128, g], F32, name=f"gp_{d}", tag="mm")
        for ec in range(NE):
            for n in range(g // 512):
                nc.tensor.matmul(
                    gp_ps[:, n * 512:(n + 1) * 512],
                    axT[:, ec * 128:(ec + 1) * 128],
                    w3[d][:, ec, n * 512:(n + 1) * 512],
                    start=(ec == 0), stop=(ec == NE - 1),
                )
        gt = sb.tile([128, g], F32, name=f"gt_{d}", tag=f"gt_{d}")
        for n in range(g // 512):
            nc.vector.tensor_add(gt[:, n * 512:(n + 1) * 512],
                                 gp_ps[:, n * 512:(n + 1) * 512],
                                 gx[d][:, n * 512:(n + 1) * 512])

        si = sb.tile([128, H], F32, name=f"si_{d}", tag="gate")
        nc.scalar.activation(si[:], gt[:, 0:H], Sig)
        tg = sb.tile([128, H], F32, name=f"tg_{d}", tag="gate")
        nc.scalar.activation(tg[:], gt[:, H:2 * H], Tanh)
        if d == "f":
            cst = sb.tile([128, H], F32, name="c_f", tag="gate")
            nc.vector.tensor_mul(cst[:], si[:], tg[:])
            tc_ = sb.tile([128, H], F32, name="tc_f", tag="gate")
            nc.scalar.activation(tc_[:], cst[:], Tanh)
            so = sb.tile([128, H], F32, name="so_f", tag="gate")
            nc.scalar.activation(so[:], gt[:, 2 * H:3 * H], Sig)
            nc.vector.tensor_mul(out_sb[:, 0:H], so[:], tc_[:])
        else:
            nc.vector.tensor_mul(out_sb[:, H:2 * H], si[:], tg[:])

    p_f, rs_f = softmax_exp("f")
    pn_f = softmax_scale("f", p_f, rs_f)
    p_b, rs_b = softmax_exp("b")
    dir_tail("f", pn_f)
    pn_b = softmax_scale("b", p_b, rs_b)
    dir_tail("b", pn_b)


def build_nc(with_attn_bias=False):
    nc = bacc.Bacc("TRN2", target_bir_lowering=False, debug=False,
                   num_devices=N_CORES)

    def din(name, shape, dt):
        return nc.dram_tensor(name, shape, dt, kind="ExternalInput").ap()

    ext = {
        "xt": din("xt", [BS, E], FP),
        "xT16": din("xT16", [E, BS], FP),
        "xtp": din("xtp", [BS, N_CORES * E], FP),
        "xnp": din("xnp", [BS, N_CORES * E], FP),
    }
    for d, g in (("f", GF), ("b", GB)):
        ext[f"mq_{d}"] = din(f"mq_{d}", [E, E], FP)
        ext[f"w3_{d}"] = din(f"w3_{d}", [E, g], FP)
        ext[f"wih_{d}"] = din(f"wih_{d}", [E, g], FP)
        ext[f"bih_{d}"] = din(f"bih_{d}", [1, g], FP)
        if with_attn_bias:
            ext[f"vecb_{d}"] = din(f"vecb_{d}", [1, B], FP)
    out_ext = nc.dram_tensor("out", [BS, 2 * H], F32, kind="ExternalOutput").ap()

    with tile.TileContext(nc) as tc:
        with (
            tc.tile_pool(name="sb", bufs=1) as sb_pool,
            tc.tile_pool(name="ps", bufs=1, space="PSUM") as ps_pool,
        ):
            class P:
                def __init__(self, pool, defaults):
                    self.pool, self.defaults = pool, defaults

                def tile(self, shape, dtype, name=None, tag=""):
                    bufs = self.defaults.get(tag, 1)
                    return self.pool.tile(shape, dtype, name=name, tag=tag,
                                          bufs=bufs)

            sb = P(sb_pool, {"w": 4, "w32": 2, "bias": 2, "gate": 6,
                             "stat": 4})
            ps = P(ps_pool, {"mm": 2, "tp": 2})

            out_sb = sb_pool.tile([BS, 2 * H], F32, name="out_sb", tag="out")

            _emit(tc, nc, sb, ps, ext, out_sb, with_attn_bias)

            nc.gpsimd.dma_start(out_ext[:], out_sb[:])

    nc.compile()
    return nc


_NC_CACHE = {}


def _get_nc(with_attn_bias=False):
    if with_attn_bias not in _NC_CACHE:
        _NC_CACHE[with_attn_bias] = build_nc(with_attn_bias)
    return _NC_CACHE[with_attn_bias]


def _prep_dir(x, Wqkv, bqkv, Wo, bo, W_ih, b_ih, b_hh, flip, want_h):
    """Host-side weight folds for one direction (shared across cores)."""
    c = np.ascontiguousarray
    Wq, Wk, Wv = Wqkv[0:E], Wqkv[E:2 * E], Wqkv[2 * E:3 * E]
    idx = np.r_[0:H, 2 * H:3 * H, 3 * H:4 * H] if want_h else np.r_[0:H, 2 * H:3 * H]
    Wih = W_ih[idx]                                     # [G, E]
    Mq = (Wq.T @ Wk) / 32.0                             # [E, E]
    W3 = Wv.T @ (Wo.T @ Wih.T)                          # [E, G]
    wih = Wih.T.copy()                                  # [E, G]
    bih = (b_ih + b_hh)[idx] + (bqkv[2 * E:3 * E] @ Wo.T + bo) @ Wih.T
    if flip:
        Mq = Mq[::-1, ::-1]
        W3 = W3[::-1, :]
        wih = wih[::-1, :]
    m = {
        "mq": c(Mq.astype(FP_NP)),
        "w3": c(W3.astype(FP_NP)),
        "wih": c(wih.astype(FP_NP)),
        "bih": c(bih.reshape(1, -1).astype(FP_NP)),
    }
    bq = bqkv[0:E]
    if np.any(bq):
        u = (Wk.T @ bq) / 32.0                          # column shift of scores
        xx = x[:, ::-1] if flip else x
        m["vecb"] = c((xx @ u).reshape(1, B).astype(FP_NP))
    return m


def _build_in_maps(inputs, Wqkv_f, bqkv_f, Wo_f, bo_f, W_ih_f, b_ih_f, b_hh_f,
                   Wqkv_b, bqkv_b, Wo_b, bo_b, W_ih_b, b_ih_b, b_hh_b):
    inputs = np.asarray(inputs, dtype=np.float32)
    x = np.ascontiguousarray(inputs[:, -1, :])           # [B, E]

    shared = {}
    for d, args in (("f", (np.asarray(Wqkv_f), np.asarray(bqkv_f),
                           np.asarray(Wo_f), np.asarray(bo_f),
                           np.asarray(W_ih_f), np.asarray(b_ih_f),
                           np.asarray(b_hh_f), False, True)),
                    ("b", (np.asarray(Wqkv_b), np.asarray(bqkv_b),
                           np.asarray(Wo_b), np.asarray(bo_b),
                           np.asarray(W_ih_b), np.asarray(b_ih_b),
                           np.asarray(b_hh_b), True, False))):
        for k, v in _prep_dir(x, *args).items():
            shared[f"{k}_{d}"] = v
    with_attn_bias = "vecb_f" in shared or "vecb_b" in shared
    if with_attn_bias:
        for d in ("f", "b"):
            if f"vecb_{d}" not in shared:
                shared[f"vecb_{d}"] = np.zeros((1, B), dtype=FP_NP)

    c = np.ascontiguousarray
    x16 = x.astype(FP_NP)
    # xtp[p, g*E + jc*128 + b] = x[g*128+b, jc*128+p]   (transposed pack)
    xtp = c(x16.reshape(N_CORES, BS, NE, 128).transpose(3, 0, 2, 1)
            .reshape(BS, N_CORES * E))
    # xnp[p, g*E + e] = x[g*128+p, e]                   (natural pack)
    xnp = c(x16.reshape(N_CORES, BS, E).transpose(1, 0, 2)
            .reshape(BS, N_CORES * E))
    shared["xtp"] = xtp
    shared["xnp"] = xnp

    in_maps = []
    for ci in range(N_CORES):
        xs = x[ci * BS:(ci + 1) * BS]                    # [128, E]
        xtpack = xs.reshape(BS, NE, 128).transpose(2, 1, 0).reshape(128, E)
        m = {"xt": c(xtpack.astype(FP_NP)),
             "xT16": c(xs.T.astype(FP_NP))}
        m.update(shared)
        in_maps.append(m)
    return in_maps, with_attn_bias


def kernel(inputs, Wqkv_f, bqkv_f, Wo_f, bo_f, W_ih_f, b_ih_f, b_hh_f,
           Wqkv_b, bqkv_b, Wo_b, bo_b, W_ih_b, b_ih_b, b_hh_b):
    in_maps, with_attn_bias = _build_in_maps(
        inputs, Wqkv_f, bqkv_f, Wo_f, bo_f, W_ih_f, b_ih_f, b_hh_f,
        Wqkv_b, bqkv_b, Wo_b, bo_b, W_ih_b, b_ih_b, b_hh_b)
    nc = _get_nc(with_attn_bias)
    res = run_bass_kernel_spmd(nc, in_maps, core_ids=list(range(N_CORES)))
    out = np.concatenate([res.results[ci]["out"] for ci in range(N_CORES)],
                         axis=0)
    return out.astype(np.float32)


# revision 39
# speedup vs baseline: 3.8002x; 1.2830x over previous
"""Trainium2 Bass kernel for nn_AttentionEnhancedBiLSTM (8 NeuronCores, SPMD).

Math (from the reference), per direction:
    x  = inputs[:, -1, :]                       # [B=1024, E=1024]
    scores = (x Wq^T)(x Wk^T)^T / 32
    af = softmax(scores) (x Wv^T) Wo^T + bo
    gates = (af + x) W_ih^T + b;  c = sig(i)*tanh(g);  h = sig(o)*tanh(c)
    out = concat([h_f, c_b], -1)   (backward direction uses xr = x[:, ::-1])

Factorization used here (exact in real arithmetic):
    scores = x Mq x^T,            Mq  = Wq^T Wk / 32
    gates  = rinv . (p @ x_all) @ W3 + x @ wih + bih_eff
        W3  = Wv^T Wo^T W_ih'^T   (W_ih' = used gate rows: fwd i,g,o; bwd i,g)
        wih = W_ih'^T
        bih_eff = (b_ih + b_hh)' + (bv Wo^T + bo) W_ih'^T   (softmax rows sum
        to 1, so bv/bo fold exactly; bk shifts scores per-row -> softmax
        invariant -> dropped; bq shifts per-column -> host vector vecb)
    The backward x-flip folds into the weights: Mq_b[::-1,::-1],
    W3_b[::-1,:], wih_b[::-1,:].

After this factorization the ONLY cross-core tensor is x itself -- an input.
So there is NO collective at all: every core receives the full x (in both
layouts, host-packed) as a replicated input, and computes its own 128 output
rows end-to-end.  q/k/v/af/lstm_in never materialize on device.

Precision: fp16 everywhere (accumulation in fp32 PSUM) except W3, which is
fp8-e4m3 pre-scaled by 16 (the scale is undone for free inside the softmax
normalization constant); measured 9.2e-3 end-to-end rel err vs the 2e-2
budget.  Softmax runs without max-subtraction: |scores| <= ~6 so exp stays
well inside fp16/fp32 range.

Sharding: batch-sharded 8 ways (128 rows/core); weights + x replicated.
"""

import numpy as np
import ml_dtypes

import concourse.bass as bass
import concourse.mybir as mybir
import concourse.tile as tile
from concourse import bacc
from concourse.bass_utils import run_bass_kernel_spmd
from concourse.masks import make_identity

N_CORES = 8
B, T, E, H = 1024, 128, 1024, 512
BS = B // N_CORES          # 128 batch rows per core
NE = E // 128              # 8 e-chunks
GF = 3 * H                 # fwd gates i,g,o
GB = 2 * H                 # bwd gates i,g
F32 = mybir.dt.float32
FP = mybir.dt.float16
FP8 = mybir.dt.float8e4
FP_NP = np.float16
FP8_NP = ml_dtypes.float8_e4m3
W3_SCALE = 16.0            # w3 pre-scaled into fp8 range; undone via rinv


def _emit(tc, nc, sb, ps, ext, out_sb, out_ext, with_attn_bias):
    Exp = mybir.ActivationFunctionType.Exp
    Sig = mybir.ActivationFunctionType.Sigmoid
    Tanh = mybir.ActivationFunctionType.Tanh
    Copy = mybir.ActivationFunctionType.Copy
    dirs = ("f", "b")
    G = {"f": GF, "b": GB}

    # ---- gathered-x loads: full x is a replicated INPUT (no collective) ---
    xTf = sb.tile([128, N_CORES * E], FP8, name="xTf", tag="xTf")
    for q in range(4):
        nc.gpsimd.dma_start(xTf[:, q * 2 * E:(q + 1) * 2 * E],
                            ext["xtp"][:, q * 2 * E:(q + 1) * 2 * E])
    xTf4 = xTf.rearrange("p (g x) -> p g x", g=N_CORES)

    # ---- local loads (sync DMA queue); xt doubles as the gx lhsT ----------
    xt = sb.tile([128, E], FP, name="xt", tag="xt")
    nc.sync.dma_start(xt[:], ext["xt"][:])

    # ---- constants ----
    identf = sb.tile([128, 128], F32, name="identf", tag="identf")
    make_identity(nc, identf[:])
    ident = sb.tile([128, 128], FP, name="ident", tag="ident")
    nc.vector.tensor_copy(ident[:], identf[:])
    onesf = sb.tile([1, 128], F32, name="onesf", tag="onesf")
    nc.gpsimd.memset(onesf[:], 1.0)
    ones = sb.tile([1, 128], FP, name="ones", tag="ones")
    nc.vector.tensor_copy(ones[:], onesf[:])

    def transpose_1024(src, name):
        """[128, 1024] fp16 natural -> chunk-transposed [128, 1024] fp16."""
        out = sb.tile([128, E], FP, name=name, tag=name)
        for half in range(2):
            tp = ps.tile([128, 512], FP, name=f"tp_{name}_{half}", tag="tp")
            for i in range(4):
                j = half * 4 + i
                nc.tensor.transpose(tp[:, i * 128:(i + 1) * 128],
                                    src[:, j * 128:(j + 1) * 128], ident[:])
            nc.vector.tensor_copy(out[:, half * 512:(half + 1) * 512], tp[:])
        return out

    # ---- xM = x @ Mq, then transpose (both dirs) --------------------------
    xmT = {}
    for d in dirs:
        xm_ps = ps.tile([128, E], F32, name=f"xm_{d}", tag="mm")
        for ec in range(NE):
            mqt = sb.tile([128, E], FP, name=f"mq_{d}_{ec}", tag="w")
            meng = nc.scalar if d == "f" else nc.sync
            meng.dma_start(mqt[:], ext[f"mq_{d}"][ec * 128:(ec + 1) * 128, :])
            for n in range(2):
                nc.tensor.matmul(
                    xm_ps[:, n * 512:(n + 1) * 512],
                    xt[:, ec * 128:(ec + 1) * 128],
                    mqt[:, n * 512:(n + 1) * 512],
                    start=(ec == 0), stop=(ec == NE - 1),
                )
        xm_sb = sb.tile([128, E], FP, name=f"xmsb_{d}", tag=f"xmsb_{d}")
        for n in range(2):
            nc.vector.tensor_copy(xm_sb[:, n * 512:(n + 1) * 512],
                                  xm_ps[:, n * 512:(n + 1) * 512])
        xmT[d] = transpose_1024(xm_sb, f"xmT_{d}")

    # ---- xnat: full x natural, host-packed fp8 (needed first at ax_f) -----
    xnat = sb.tile([128, N_CORES * E], FP8, name="xnat", tag="xnat")
    for q in range(4):
        nc.sync.dma_start(xnat[:, q * 2 * E:(q + 1) * 2 * E],
                          ext["xnp"][:, q * 2 * E:(q + 1) * 2 * E])

    # ---- gx = x @ wih + bih  (both dirs; per-512 psum pieces so the tail's
    # psum slots never wait on the wih-paced gx copies) ----------------------
    gx = {}
    wih_sb = {}
    bih_sb = {}
    for d in dirs:
        g = G[d]
        quarters = []
        for q in range(4):                     # quarter-matrix wih slices
            wt = sb.tile([128, 2 * g], FP, name=f"wih_{d}_{q}", tag="w32")
            if d == "f":
                weng = nc.sync if q < 2 else nc.scalar
            else:
                weng = nc.gpsimd
            weng.dma_start(wt[:], ext[f"wih_{d}"][:, 2 * q * g:
                                                  (2 * q + 2) * g])
            quarters.append(wt)
        wih_sb[d] = quarters
        bih = sb.tile([1, g], FP, name=f"bih_{d}", tag="bias")
        (nc.sync if d == "f" else nc.gpsimd).dma_start(bih[:],
                                                       ext[f"bih_{d}"][:])
        bih_sb[d] = bih

    def emit_gx(d):
        g = G[d]
        gx_sb = sb.tile([128, g], F32, name=f"gx_{d}", tag=f"gx_{d}")
        for k in range(g // 512):
            gp = ps.tile([128, 512], F32, name=f"gxps_{d}_{k}", tag="gxp")
            for ec in range(NE):
                wt = wih_sb[d][ec // 2]
                sub = ec % 2
                nc.tensor.matmul(
                    gp[:], xt[:, ec * 128:(ec + 1) * 128],
                    wt[:, sub * g + k * 512: sub * g + (k + 1) * 512],
                    start=(ec == 0), stop=False,
                )
            nc.tensor.matmul(
                gp[:], ones[0:1, :],
                bih_sb[d][0:1, k * 512:(k + 1) * 512],
                start=False, stop=True,
            )
            nc.vector.tensor_copy(gx_sb[:, k * 512:(k + 1) * 512], gp[:])
        gx[d] = gx_sb

    # ---- w3 (whole matrices, needed only at gp) ---------------------------
    w3 = {}
    for d in dirs:
        g = G[d]
        w3_sb = sb.tile([128, NE * g], FP8, name=f"w3_{d}", tag=f"w3_{d}")
        eng = nc.scalar if d == "f" else nc.gpsimd
        eng.dma_start(w3_sb[:], ext[f"w3_{d}"][:])
        w3[d] = w3_sb.rearrange("p (n m) -> p n m", n=NE)

    # ---- scores for both dirs (before gx: their psum slots chain off xm,
    # and the PE reaches them as soon as xmT is ready) ----------------------
    sc_ps = {}
    for d in dirs:
        acc = ps.tile([128, B], F32, name=f"sc_{d}", tag="mm")
        for jc in range(NE):
            for n in range(B // 512):
                nc.tensor.matmul(
                    acc[:, n * 512:(n + 1) * 512],
                    xmT[d][:, jc * 128:(jc + 1) * 128],
                    xTf4[:, 4 * n:4 * (n + 1), jc * 128:(jc + 1) * 128],
                    start=(jc == 0),
                    stop=(jc == NE - 1 and not with_attn_bias),
                )
        if with_attn_bias:
            vb = sb.tile([1, B], FP, name=f"vecb_{d}", tag="bias")
            nc.sync.dma_start(vb[:], ext[f"vecb_{d}"][:])
            for n in range(B // 512):
                nc.tensor.matmul(
                    acc[:, n * 512:(n + 1) * 512],
                    ones[0:1, :],
                    vb[0:1, n * 512:(n + 1) * 512],
                    start=False, stop=(n == B // 512 - 1),
                )
        sc_ps[d] = acc

    emit_gx("f")
    emit_gx("b")

    # ---- softmax (no max-subtraction; |scores| <= ~6) + per-dir tail ------
    def softmax_exp(d):
        rowsum = sb.tile([128, 1], F32, name=f"rowsum_{d}", tag="stat")
        p_sb = sb.tile([128, B], FP, name=f"p_{d}", tag=f"p_{d}")
        nc.scalar.activation(p_sb[:], sc_ps[d][:], Exp, accum_out=rowsum[:])
        return p_sb, rowsum

    def softmax_scale(d, p_sb, rowsum):
        rinv = sb.tile([128, 1], F32, name=f"rinv_{d}", tag="stat")
        nc.vector.reciprocal(rinv[:], rowsum[:])
        rinv2 = sb.tile([128, 1], F32, name=f"rinv2_{d}", tag="stat")
        nc.vector.tensor_scalar_mul(rinv2[:], rinv[:], 1.0 / W3_SCALE)
        pn_sb = sb.tile([128, B], FP, name=f"pn_{d}", tag=f"pn_{d}")
        for n in range(2):
            nc.vector.tensor_scalar_mul(pn_sb[:, n * 512:(n + 1) * 512],
                                        p_sb[:, n * 512:(n + 1) * 512],
                                        rinv2[:])
        return pn_sb

    def dir_tail(d, pn_sb):
        g = G[d]
        pT = transpose_1024(pn_sb, f"pT_{d}")
        ax_ps = ps.tile([128, E], F32, name=f"ax_{d}", tag="mm")
        for bc in range(NE):
            for n in range(2):
                nc.tensor.matmul(
                    ax_ps[:, n * 512:(n + 1) * 512],
                    pT[:, bc * 128:(bc + 1) * 128],
                    xnat[:, bc * E + n * 512: bc * E + (n + 1) * 512],
                    start=(bc == 0), stop=(bc == NE - 1),
                )
        ax_sb = sb.tile([128, E], FP, name=f"axsb_{d}", tag=f"axsb_{d}")
        for n in range(2):
            nc.scalar.activation(ax_sb[:, n * 512:(n + 1) * 512],
                                 ax_ps[:, n * 512:(n + 1) * 512], Copy)
        axT = transpose_1024(ax_sb, f"axT_{d}")

        gt = sb.tile([128, g], F32, name=f"gt_{d}", tag=f"gt_{d}")
        for k in range(g // 512):
            gp_ps = ps.tile([128, 512], F32, name=f"gp_{d}_{k}", tag="gxp")
            for ec in range(NE):
                nc.tensor.matmul(
                    gp_ps[:],
                    axT[:, ec * 128:(ec + 1) * 128],
                    w3[d][:, ec, k * 512:(k + 1) * 512],
                    start=(ec == 0), stop=(ec == NE - 1),
                )
            nc.vector.tensor_add(gt[:, k * 512:(k + 1) * 512],
                                 gp_ps[:],
                                 gx[d][:, k * 512:(k + 1) * 512])

        si = sb.tile([128, H], F32, name=f"si_{d}", tag="gate")
        nc.scalar.activation(si[:], gt[:, 0:H], Sig)
        tg = sb.tile([128, H], F32, name=f"tg_{d}", tag="gate")
        nc.scalar.activation(tg[:], gt[:, H:2 * H], Tanh)
        if d == "f":
            cst = sb.tile([128, H], F32, name="c_f", tag="gate")
            nc.vector.tensor_mul(cst[:], si[:], tg[:])
            tc_ = sb.tile([128, H], F32, name="tc_f", tag="gate")
            nc.scalar.activation(tc_[:], cst[:], Tanh)
            so = sb.tile([128, H], F32, name="so_f", tag="gate")
            nc.scalar.activation(so[:], gt[:, 2 * H:3 * H], Sig)
            nc.vector.tensor_mul(out_sb[:, 0:H], so[:], tc_[:])
            nc.gpsimd.dma_start(out_ext[:, 0:H], out_sb[:, 0:H])
        else:
            nc.vector.tensor_mul(out_sb[:, H:2 * H], si[:], tg[:])
            nc.gpsimd.dma_start(out_ext[:, H:2 * H], out_sb[:, H:2 * H])

    p_f, rs_f = softmax_exp("f")
    pn_f = softmax_scale("f", p_f, rs_f)
    p_b, rs_b = softmax_exp("b")
    dir_tail("f", pn_f)
    pn_b = softmax_scale("b", p_b, rs_b)
    dir_tail("b", pn_b)


def build_nc(with_attn_bias=False):
    nc = bacc.Bacc("TRN2", target_bir_lowering=False, debug=False,
                   num_devices=N_CORES)

    def din(name, shape, dt):
        return nc.dram_tensor(name, shape, dt, kind="ExternalInput").ap()

    ext = {
        "xt": din("xt", [BS, E], FP),
        "xtp": din("xtp", [BS, N_CORES * E], FP8),
        "xnp": din("xnp", [BS, N_CORES * E], FP8),
    }
    for d, g in (("f", GF), ("b", GB)):
        ext[f"mq_{d}"] = din(f"mq_{d}", [E, E], FP)
        ext[f"w3_{d}"] = din(f"w3_{d}", [BS, NE * g], FP8)
        ext[f"wih_{d}"] = din(f"wih_{d}", [BS, NE * g], FP)
        ext[f"bih_{d}"] = din(f"bih_{d}", [1, g], FP)
        if with_attn_bias:
            ext[f"vecb_{d}"] = din(f"vecb_{d}", [1, B], FP)
    out_ext = nc.dram_tensor("out", [BS, 2 * H], F32, kind="ExternalOutput").ap()

    with tile.TileContext(nc) as tc:
        with (
            tc.tile_pool(name="sb", bufs=1) as sb_pool,
            tc.tile_pool(name="ps", bufs=1, space="PSUM") as ps_pool,
        ):
            class P:
                def __init__(self, pool, defaults):
                    self.pool, self.defaults = pool, defaults

                def tile(self, shape, dtype, name=None, tag=""):
                    bufs = self.defaults.get(tag, 1)
                    return self.pool.tile(shape, dtype, name=name, tag=tag,
                                          bufs=bufs)

            sb = P(sb_pool, {"w": 8, "w32": 4, "bias": 2, "gate": 6,
                             "stat": 4})
            ps = P(ps_pool, {"mm": 2, "tp": 2, "gxp": 2})

            out_sb = sb_pool.tile([BS, 2 * H], F32, name="out_sb", tag="out")

            _emit(tc, nc, sb, ps, ext, out_sb, out_ext, with_attn_bias)

    nc.compile()
    return nc


_NC_CACHE = {}


def _get_nc(with_attn_bias=False):
    if with_attn_bias not in _NC_CACHE:
        _NC_CACHE[with_attn_bias] = build_nc(with_attn_bias)
    return _NC_CACHE[with_attn_bias]


def _prep_dir(x, Wqkv, bqkv, Wo, bo, W_ih, b_ih, b_hh, flip, want_h):
    """Host-side weight folds for one direction (shared across cores)."""
    c = np.ascontiguousarray
    Wq, Wk, Wv = Wqkv[0:E], Wqkv[E:2 * E], Wqkv[2 * E:3 * E]
    idx = np.r_[0:H, 2 * H:3 * H, 3 * H:4 * H] if want_h else np.r_[0:H, 2 * H:3 * H]
    Wih = W_ih[idx]                                     # [G, E]
    Mq = (Wq.T @ Wk) / 32.0                             # [E, E]
    W3 = Wv.T @ (Wo.T @ Wih.T)                          # [E, G]
    wih = Wih.T.copy()                                  # [E, G]
    bih = (b_ih + b_hh)[idx] + (bqkv[2 * E:3 * E] @ Wo.T + bo) @ Wih.T
    if flip:
        Mq = Mq[::-1, ::-1]
        W3 = W3[::-1, :]
        wih = wih[::-1, :]
    def pmajor(w):   # [E, g] -> [128, NE*g], row p holds chunks n contiguously
        g_ = w.shape[1]
        return w.reshape(NE, 128, g_).transpose(1, 0, 2).reshape(128, NE * g_)

    m = {
        "mq": c(Mq.astype(FP_NP)),
        "w3": c(pmajor(W3 * W3_SCALE).astype(FP8_NP)),
        "wih": c(pmajor(wih).astype(FP_NP)),
        "bih": c(bih.reshape(1, -1).astype(FP_NP)),
    }
    bq = bqkv[0:E]
    if np.any(bq):
        u = (Wk.T @ bq) / 32.0                          # column shift of scores
        xx = x[:, ::-1] if flip else x
        m["vecb"] = c((xx @ u).reshape(1, B).astype(FP_NP))
    return m


def _build_in_maps(inputs, Wqkv_f, bqkv_f, Wo_f, bo_f, W_ih_f, b_ih_f, b_hh_f,
                   Wqkv_b, bqkv_b, Wo_b, bo_b, W_ih_b, b_ih_b, b_hh_b):
    inputs = np.asarray(inputs, dtype=np.float32)
    x = np.ascontiguousarray(inputs[:, -1, :])           # [B, E]

    shared = {}
    for d, args in (("f", (np.asarray(Wqkv_f), np.asarray(bqkv_f),
                           np.asarray(Wo_f), np.asarray(bo_f),
                           np.asarray(W_ih_f), np.asarray(b_ih_f),
                           np.asarray(b_hh_f), False, True)),
                    ("b", (np.asarray(Wqkv_b), np.asarray(bqkv_b),
                           np.asarray(Wo_b), np.asarray(bo_b),
                           np.asarray(W_ih_b), np.asarray(b_ih_b),
                           np.asarray(b_hh_b), True, False))):
        for k, v in _prep_dir(x, *args).items():
            shared[f"{k}_{d}"] = v
    with_attn_bias = "vecb_f" in shared or "vecb_b" in shared
    if with_attn_bias:
        for d in ("f", "b"):
            if f"vecb_{d}" not in shared:
                shared[f"vecb_{d}"] = np.zeros((1, B), dtype=FP_NP)

    c = np.ascontiguousarray
    x16 = x.astype(FP_NP)
    # xtp[p, g*E + jc*128 + b] = x[g*128+b, jc*128+p]   (transposed pack)
    xtp = c(x.reshape(N_CORES, BS, NE, 128).transpose(3, 0, 2, 1)
            .reshape(BS, N_CORES * E).astype(FP8_NP))
    # xnp[p, g*E + e] = x[g*128+p, e]                   (natural pack)
    xnp = c(x.reshape(N_CORES, BS, E).transpose(1, 0, 2)
            .reshape(BS, N_CORES * E).astype(FP8_NP))
    shared["xtp"] = xtp
    shared["xnp"] = xnp

    in_maps = []
    for ci in range(N_CORES):
        xs = x[ci * BS:(ci + 1) * BS]                    # [128, E]
        xtpack = xs.reshape(BS, NE, 128).transpose(2, 1, 0).reshape(128, E)
        m = {"xt": c(xtpack.astype(FP_NP))}
        m.update(shared)
        in_maps.append(m)
    return in_maps, with_attn_bias


def kernel(inputs, Wqkv_f, bqkv_f, Wo_f, bo_f, W_ih_f, b_ih_f, b_hh_f,
           Wqkv_b, bqkv_b, Wo_b, bo_b, W_ih_b, b_ih_b, b_hh_b):
    in_maps, with_attn_bias = _build_in_maps(
        inputs, Wqkv_f, bqkv_f, Wo_f, bo_f, W_ih_f, b_ih_f, b_hh_f,
        Wqkv_b, bqkv_b, Wo_b, bo_b, W_ih_b, b_ih_b, b_hh_b)
    nc = _get_nc(with_attn_bias)
    res = run_bass_kernel_spmd(nc, in_maps, core_ids=list(range(N_CORES)))
    out = np.concatenate([res.results[ci]["out"] for ci in range(N_CORES)],
                         axis=0)
    return out.astype(np.float32)
